# revision 27
# baseline (speedup 1.0000x reference)
"""GNN message-passing (GCN-mean) kernel for 8 Trainium2 NeuronCores. (V3)

Like V2.2 (batched gathers per superblock, bf16 one-hot matmul scatter,
fused epilogue) plus DIAGONAL SLOTTING: per (block, src-core), the first K0
edges of each dst land at partition dst_rel of K0 "diagonal" chunks, so
their aggregation is a plain identity matmul (no one-hot build); only
overflow edges (beyond K0 per dst) use one-hot chunks.  Pad slots gather
the zero row (SHP-1), which is kept zero in every layer's ag_in.
"""
import math
import os
import numpy as np

NC = 8
P = 128
EPS = 1e-5
SB = 7        # dst blocks per superblock
NCHUNK = 7
K0 = int(os.environ.get("K0", "2"))  # diagonal chunks per (block, group)

_CACHE = {}
_LAST_EXEC = None


# --------------------------------------------------------------------------
# device program
# --------------------------------------------------------------------------
def _build_program(NB, SHP, LAYERS, OUT, meta):
    import concourse.bacc as bacc
    import concourse.mybir as mybir
    import concourse.tile as tile
    from concourse.masks import make_identity

    NSUP = NB // SB
    SUPC = NSUP // NCHUNK
    BLKC = NB // NCHUNK
    XCOLS = meta["XCOLS"]
    DCOLS = meta["DCOLS"]
    IDXC = meta["IDXC"]
    OHMAX = meta["OHMAX"]
    per_sg = meta["per_sg"]          # [NSUP, 8] chunks per gather
    g_off = meta["g_off"]            # [NSUP, 8]
    reg_off = meta["reg_off"]        # [NSUP, SB, 8] diag col base
    pairs = meta["pairs"]            # [s][bl] -> [(g, xg col, dst col), ...]
    f32 = mybir.dt.float32
    bf16 = mybir.dt.bfloat16
    Alu = mybir.AluOpType
    Act = mybir.ActivationFunctionType

    nc = bacc.Bacc("TRN2", target_bir_lowering=False, num_devices=NC,
                   num_swdge_queues=4)

    nodes_bf = nc.declare_dram_parameter("nodes_bf", [P, SHP], bf16, isOutput=False)
    idxs = nc.declare_dram_parameter("idxs", [NSUP, P, IDXC], mybir.dt.int16, isOutput=False)
    dstrel = nc.declare_dram_parameter("dstrel", [NSUP, P, DCOLS], bf16, isOutput=False)
    invp = nc.declare_dram_parameter("invp", [P, NB], f32, isOutput=False)
    deg1p = nc.declare_dram_parameter("deg1p", [1, NB * P], bf16, isOutput=False)
    maskp = nc.declare_dram_parameter("maskp", [P, 1], f32, isOutput=False)
    W_in = nc.declare_dram_parameter("W_in", [P, P], bf16, isOutput=False)
    Wl = nc.declare_dram_parameter("Wl", [LAYERS, P, P], bf16, isOutput=False)
    Wout = nc.declare_dram_parameter("Wout", [P, OUT], bf16, isOutput=False)
    b_in_r = nc.declare_dram_parameter("b_in_r", [1, P], bf16, isOutput=False)
    bl_r = nc.declare_dram_parameter("bl_r", [LAYERS, 1, P], bf16, isOutput=False)
    bout_r = nc.declare_dram_parameter("bout_r", [1, OUT], bf16, isOutput=False)
    iota_in = nc.declare_dram_parameter("iota", [P, P], bf16, isOutput=False)
    out_own = nc.declare_dram_parameter("out_own", [SHP, OUT], f32, isOutput=True)

    rg = [list(range(NC))]

    with tile.TileContext(nc) as tc:
        with (
            tc.tile_pool(name="const", bufs=1) as cpool,
            tc.tile_pool(name="dram", bufs=1, space="DRAM") as dpool,
            tc.tile_pool(name="work", bufs=6) as wpool,
            tc.tile_pool(name="ohp", bufs=4) as ohpool,
            tc.tile_pool(name="idxp", bufs=3) as ipool,
            tc.tile_pool(name="xgp", bufs=2) as xgpool,
            tc.tile_pool(name="psum", bufs=3, space="PSUM") as ppool,
            tc.tile_pool(name="psum1", bufs=1, space="PSUM") as ppool1,
        ):
            iota_t = cpool.tile([P, P], bf16)
            nc.sync.dma_start(out=iota_t[:], in_=iota_in[:])
            ident_bf = cpool.tile([P, P], bf16)
            make_identity(nc, ident_bf[:])
            eps_t = cpool.tile([P, 1], f32)
            nc.vector.memset(eps_t[:], EPS)
            W_in_t = cpool.tile([P, P], bf16)
            nc.sync.dma_start(out=W_in_t[:], in_=W_in[:])
            Wout_t = cpool.tile([P, OUT], bf16)
            nc.sync.dma_start(out=Wout_t[:], in_=Wout[:])
            ones1 = cpool.tile([1, P], bf16)
            nc.vector.memset(ones1[:], 1.0)
            bin_t = cpool.tile([1, P], bf16)
            nc.sync.dma_start(out=bin_t[:], in_=b_in_r[:])
            bl_t = []
            for l in range(LAYERS):
                t = cpool.tile([1, P], bf16, name=f"bl{l}")
                nc.sync.dma_start(out=t[:], in_=bl_r[l])
                bl_t.append(t)
            Wl_ts = []
            for l in range(LAYERS):
                t = cpool.tile([P, P], bf16, name=f"wl{l}")
                nc.sync.dma_start(out=t[:], in_=Wl[l])
                Wl_ts.append(t)
            bout_t = cpool.tile([1, OUT], bf16)
            nc.sync.dma_start(out=bout_t[:], in_=bout_r[:])
            inv_t = cpool.tile([P, NB], f32)
            nc.sync.dma_start(out=inv_t[:], in_=invp[:])
            mask_t = cpool.tile([P, 1], f32)
            nc.sync.dma_start(out=mask_t[:], in_=maskp[:])

            ag_in = [dpool.tile([SHP, P], bf16, name=f"ag_in{l}") for l in range(LAYERS)]
            x_full = [
                dpool.tile([NC, SHP, P], bf16, addr_space="Shared", name=f"x_full{l}")
                for l in range(LAYERS)
            ]

            def emit_ag(l_t, k):
                nc.gpsimd.collective_compute(
                    "AllGather", Alu.bypass, replica_groups=rg,
                    ins=[ag_in[l_t][:].opt()],
                    outs=[x_full[l_t][:].opt()],
                )

            # ---------------- Phase A: x0 = nodes @ W_in + b_in ----------
            for b in range(NB):
                nbT = wpool.tile([P, P], bf16, tag="nb")
                nc.sync.dma_start(out=nbT[:], in_=nodes_bf[:, b * P:(b + 1) * P])
                ph = ppool.tile([P, P], f32, tag="h", space="PSUM")
                nc.tensor.matmul(ph[:], lhsT=nbT[:], rhs=W_in_t[:], start=True, stop=False)
                nc.tensor.matmul(ph[:], lhsT=ones1[:], rhs=bin_t[:], start=False, stop=True)
                x0b = wpool.tile([P, P], bf16, tag="xnext")
                if b == NB - 1:
                    # zero the pad node rows so the gather zero-row stays 0
                    nc.scalar.activation(x0b[:], ph[:], Act.Copy, scale=mask_t[:, 0:1])
                else:
                    nc.scalar.copy(out=x0b[:], in_=ph[:])
                nc.sync.dma_start(out=ag_in[0][b * P:(b + 1) * P, :], in_=x0b[:])
            emit_ag(0, 0)

            # ---------------- Layers ------------------------------------
            for l in range(LAYERS):
                xf = x_full[l]
                last = l == LAYERS - 1
                for s in range(NSUP):
                    idx_t = ipool.tile([P, IDXC], mybir.dt.int16, tag="idx")
                    nc.sync.dma_start(out=idx_t[:], in_=idxs[s])
                    dst_t = ipool.tile([P, DCOLS], bf16, tag="dst")
                    nc.sync.dma_start(out=dst_t[:], in_=dstrel[s])
                    deg1_t = ipool.tile([1, SB * P], bf16, tag="deg1")
                    nc.sync.dma_start(out=deg1_t[:],
                                      in_=deg1p[0:1, s * SB * P:(s + 1) * SB * P])
                    xg = xgpool.tile([P, XCOLS, P], bf16, tag="xg")
                    for g in range(8):
                        nci = int(per_sg[s, g]) * P
                        co = int(g_off[s, g])
                        nc.gpsimd.dma_gather(
                            out_ap=xg[:, co:co + int(per_sg[s, g]), :],
                            in_ap=xf[g],
                            idxs_ap=idx_t[:, co * 8:co * 8 + nci // 16],
                            num_idxs=nci,
                            num_idxs_reg=nci,
                            elem_size=P,
                            single_packet=False,
                            queue_num=g % 4,
                        )
                    def build_oh(bl):
                        # one batched DVE op builds all of a block's one-hots
                        prs_b = pairs[s][bl]
                        t = ohpool.tile([P, OHMAX, P], bf16, tag="oh")
                        if prs_b:
                            noh_b = len(prs_b)
                            dc0 = prs_b[0][2]
                            nc.vector.tensor_tensor(
                                out=t[:, 0:noh_b, :],
                                in0=iota_t[:].unsqueeze(1).to_broadcast([P, noh_b, P]),
                                in1=dst_t[:, dc0:dc0 + noh_b].unsqueeze(2)
                                    .to_broadcast([P, noh_b, P]),
                                op=Alu.is_equal,
                            )
                        return t

                    oh_q = [build_oh(0), build_oh(1)]
                    for bl_i in range(SB):
                        b = s * SB + bl_i
                        prs = pairs[s][bl_i]   # [(g, xg col, dst col), ...]
                        oh = oh_q.pop(0)
                        if bl_i + 2 < SB:
                            oh_q.append(build_oh(bl_i + 2))
                        xs_bf = wpool.tile([P, P], bf16, tag="xs")
                        nc.sync.dma_start(out=xs_bf[:], in_=ag_in[l][b * P:(b + 1) * P, :])
                        noh = len(prs)
                        # aggregate TRANSPOSED: paggT[feat, dst] avoids the
                        # per-block PE transpose + ACT copy before the W matmul
                        paggT = ppool.tile([P, P], f32, tag="agg", space="PSUM")
                        nmm = 1 + 8 * K0 + noh
                        nc.tensor.matmul(paggT[:], lhsT=xs_bf[:], rhs=ident_bf[:],
                                         start=True, stop=(nmm == 1))
                        mm = 1
                        for g in range(8):
                            ro = int(reg_off[s, bl_i, g])
                            for k in range(K0):
                                mm += 1
                                nc.tensor.matmul(
                                    paggT[:], lhsT=xg[:, ro + k, :], rhs=ident_bf[:],
                                    start=False, stop=(mm == nmm),
                                )
                        for j, (g_, col_, dc_) in enumerate(prs):
                            mm += 1
                            nc.tensor.matmul(
                                paggT[:], lhsT=xg[:, col_, :], rhs=oh[:, j, :],
                                start=False, stop=(mm == nmm),
                            )
                        # ---- fused epilogue ----
                        m1T = wpool.tile([P, P], bf16, tag="m1")
                        nc.scalar.copy(out=m1T[:], in_=paggT[:])
                        ph = ppool.tile([P, P], f32, tag="h", space="PSUM")
                        nc.tensor.matmul(ph[:], lhsT=m1T[:], rhs=Wl_ts[l][:],
                                         start=True, stop=False)
                        # bias as outer((deg+1), b): inv scale below folds it to +b
                        nc.tensor.matmul(ph[:], lhsT=deg1_t[0:1, bl_i * P:(bl_i + 1) * P],
                                         rhs=bl_t[l][:], start=False, stop=True)
                        hr = wpool.tile([P, P], f32, tag="hr")
                        mu_s = wpool.tile([P, 1], f32, tag="mus")
                        nc.scalar.activation(hr[:], ph[:], Act.Relu,
                                             scale=inv_t[:, b:b + 1], accum_out=mu_s[:])
                        h2 = wpool.tile([P, P], f32, tag="h2")
                        s2 = wpool.tile([P, 1], f32, tag="s2")
                        nc.scalar.activation(h2[:], hr[:], Act.Square, accum_out=s2[:])
                        musq = wpool.tile([P, 1], f32, tag="musq")
                        nc.scalar.activation(musq[:], mu_s[:], Act.Square,
                                             scale=1.0 / P)
                        var2 = wpool.tile([P, 1], f32, tag="var2")
                        nc.vector.scalar_tensor_tensor(
                            out=var2[:], in0=s2[:], scalar=1.0 / P, in1=musq[:],
                            op0=Alu.mult, op1=Alu.subtract,
                        )
                        std_t = wpool.tile([P, 1], f32, tag="std")
                        nc.scalar.activation(std_t[:], var2[:], Act.Sqrt, bias=eps_t[:, 0:1])
                        rstd = wpool.tile([P, 1], f32, tag="rstd")
                        nc.vector.reciprocal_approx_fast(out=rstd[:], in_=std_t[:])
                        nmr = wpool.tile([P, 1], f32, tag="nmr")
                        nc.vector.tensor_scalar(
                            out=nmr[:], in0=mu_s[:], scalar1=rstd[:, 0:1],
                            scalar2=-1.0 / P, op0=Alu.mult, op1=Alu.mult,
                        )
                        y_bf = wpool.tile([P, P], bf16, tag="xnext" if not last else "yf")
                        nc.scalar.activation(y_bf[:], hr[:], Act.Identity,
                                             scale=rstd[:, 0:1], bias=nmr[:, 0:1])
                        if not last:
                            if b == NB - 1:
                                ym = wpool.tile([P, P], bf16, tag="ym")
                                nc.vector.tensor_scalar(
                                    out=ym[:], in0=y_bf[:], scalar1=mask_t[:, 0:1],
                                    scalar2=None, op0=Alu.mult,
                                )
                                nc.sync.dma_start(out=ag_in[l + 1][b * P:(b + 1) * P, :], in_=ym[:])
                            else:
                                nc.sync.dma_start(out=ag_in[l + 1][b * P:(b + 1) * P, :], in_=y_bf[:])
                        else:
                            pyT = ppool1.tile([P, P], bf16, tag="mT", space="PSUM")
                            nc.tensor.transpose(pyT[:], y_bf[:], ident_bf[:])
                            yT = wpool.tile([P, P], bf16, tag="mTs")
                            nc.scalar.copy(out=yT[:], in_=pyT[:])
                            po = ppool1.tile([P, OUT], f32, tag="po", space="PSUM")
                            nc.tensor.matmul(po[:], lhsT=yT[:], rhs=Wout_t[:], start=True, stop=False)
                            nc.tensor.matmul(po[:], lhsT=ones1[:], rhs=bout_t[:], start=False, stop=True)
                            ob = wpool.tile([P, OUT], f32, tag="ob")
                            nc.scalar.copy(out=ob[:], in_=po[:])
                            nc.sync.dma_start(out=out_own[b * P:(b + 1) * P, :], in_=ob[:])
                if not last:
                    emit_ag(l + 1, 0)

    nc.compile()
    return nc


# --------------------------------------------------------------------------
# host-side sharding prep (diagonal slotting)
# --------------------------------------------------------------------------
def _prep_edges(src, dst, N, SH, SHP, NB):
    E = src.shape[0]
    src = src.astype(np.int64)
    dst = dst.astype(np.int64)
    NSUP = NB // SB
    ZR = SHP - 1  # zero row within each group table

    core = dst // SH
    dst_loc = dst - core * SH
    blk = dst_loc >> 7
    dst_rel = dst_loc & 127
    grp = src // SH
    src_loc = src - grp * SH

    key = ((core * NB + blk) * 8 + grp) * P + dst_rel
    order = np.lexsort((src_loc, key))
    ks = key[order]
    sl = src_loc[order]
    dr = dst_rel[order]

    cell_counts = np.bincount(ks, minlength=NC * NB * 8 * P)
    cell_starts = np.zeros_like(cell_counts)
    np.cumsum(cell_counts[:-1], out=cell_starts[1:])
    rank = np.arange(E, dtype=np.int64) - cell_starts[ks]

    cbg = ks // P
    d = ks % P

    is_diag = rank < K0
    ovf_mask = ~is_diag

    # overflow edges pooled per (superblock, group) with STATIC per-block
    # ranges (max over cores) so chunk->block structure is core-independent
    ovf_counts = np.bincount(cbg[ovf_mask], minlength=NC * NB * 8).reshape(NC, NB, 8)
    R_sb = ovf_counts.max(axis=0).reshape(NSUP, SB, 8)   # static range [s, bl, g]
    start_off = np.zeros((NSUP, SB, 8), np.int64)
    start_off[:, 1:, :] = np.cumsum(R_sb, axis=1)[:, :-1, :]
    pool = R_sb.sum(axis=1)                              # [s, g]
    novf_chunks = -(-pool // P)                          # [s, g]

    per_sg = SB * K0 + novf_chunks                       # [s, g] cols per gather
    g_off = np.zeros((NSUP, 8), np.int64)
    g_off[:, 1:] = np.cumsum(per_sg[:, :-1], axis=1)
    reg_off = (g_off[:, None, :] +
               (np.arange(SB) * K0)[None, :, None])      # diag col base [s, bl, g]
    ovf_base = g_off + SB * K0                           # first ovf col [s, g]
    XCOLS = int(per_sg.sum(axis=1).max())

    # (chunk, block) pair lists per (s, bl): static from ranges
    pairs = [[[] for _ in range(SB)] for _ in range(NSUP)]
    maxci = int(novf_chunks.max())
    dcol_lut = np.full((NSUP, 8, maxci, SB), -1, np.int64)
    DCOLS = 0
    for s in range(NSUP):
        dcol = 0
        for bl in range(SB):
            for gg in range(8):
                rn = int(R_sb[s, bl, gg])
                if rn == 0:
                    continue
                st = int(start_off[s, bl, gg])
                for ci in range(st >> 7, ((st + rn - 1) >> 7) + 1):
                    pairs[s][bl].append((gg, int(ovf_base[s, gg] + ci), dcol))
                    dcol_lut[s, gg, ci, bl] = dcol
                    dcol += 1
        DCOLS = max(DCOLS, dcol)
    OHMAX = max(len(pairs[s][bl]) for s in range(NSUP) for bl in range(SB))

    sup_e = (cbg // 8) % NB // SB
    b_local = ((cbg // 8) % NB) % SB
    g = cbg % 8
    c_ = cbg // (NB * 8)

    ovf_rank = np.zeros(E, np.int64)
    oc = np.bincount(cbg[ovf_mask], minlength=NC * NB * 8)
    os_ = np.zeros_like(oc)
    np.cumsum(oc[:-1], out=os_[1:])
    # rank overflow edges within their (core, block, group) pool in
    # src-ascending order for HBM gather locality
    ovf_idx = np.nonzero(ovf_mask)[0]
    o2 = np.lexsort((sl[ovf_idx], cbg[ovf_idx]))
    sorted_pos = ovf_idx[o2]
    ovf_rank[sorted_pos] = np.arange(len(ovf_idx)) - os_[cbg[sorted_pos]]

    pool_pos = start_off[sup_e, b_local, g] + ovf_rank
    chunk = np.where(
        is_diag,
        reg_off[sup_e, b_local, g] + rank,
        ovf_base[sup_e, g] + (pool_pos >> 7),
    )
    part = np.where(is_diag, d, pool_pos & 127)

    IDX_TOT = XCOLS * P
    # pad slots gather zero rows; spread across all SHP-SH zero rows to avoid
    # HBM same-address hotspotting (same-row descriptors measured ~7x slower)
    NZ = SHP - SH
    padpat = (SH + (np.arange(IDX_TOT) % NZ)).astype(np.int16)
    idx16 = np.broadcast_to(padpat, (NC, NSUP, IDX_TOT)).copy()
    slot = chunk * P + part
    idx16[c_, sup_e, slot] = sl.astype(np.int16)

    dstv = np.full((NC, NSUP, DCOLS, P), -1.0, np.float32)
    m = ovf_mask
    ocol = dcol_lut[sup_e[m], g[m], pool_pos[m] >> 7, b_local[m]]
    assert (ocol >= 0).all()
    dstv[c_[m], sup_e[m], ocol, part[m]] = dr[m].astype(np.float32)
    import ml_dtypes
    dst_dev = np.ascontiguousarray(dstv.transpose(0, 1, 3, 2)).astype(ml_dtypes.bfloat16)

    IDXC = IDX_TOT // 16
    A = idx16.reshape(NC, NSUP, IDXC, 16)
    Bm = A.transpose(0, 1, 3, 2)
    idx_dev = np.ascontiguousarray(
        np.broadcast_to(Bm[:, :, None, :, :], (NC, NSUP, 8, 16, IDXC))
        .reshape(NC, NSUP, P, IDXC)
    )
    meta = dict(caps=R_sb, per_sg=per_sg, g_off=g_off, reg_off=reg_off,
                pairs=pairs, XCOLS=XCOLS,
                DCOLS=DCOLS, IDXC=IDXC, OHMAX=OHMAX)
    return idx_dev, dst_dev, meta


def _run(nc_prog, in_maps):
    import jax
    import numpy as np
    from jax.sharding import Mesh, PartitionSpec, NamedSharding
    from jax.experimental.shard_map import shard_map
    import concourse.mybir as mybir
    from concourse.bass2jax import _bass_exec_p, install_neuronx_cc_hook, partition_id_tensor

    install_neuronx_cc_hook()
    nc = nc_prog
    partition_name = nc.partition_id_tensor.name if nc.partition_id_tensor else None
    in_names, out_names, out_avals, zero_outs = [], [], [], []
    for alloc in nc.m.functions[0].allocations:
        if not isinstance(alloc, mybir.MemoryLocationSet):
            continue
        name = alloc.memorylocations[0].name
        if alloc.kind == "ExternalInput":
            if name != partition_name:
                in_names.append(name)
        elif alloc.kind == "ExternalOutput":
            out_names.append(name)
            shape = tuple(alloc.tensor_shape)
            dtype = mybir.dt.np(alloc.dtype)
            out_avals.append(jax.core.ShapedArray(shape, dtype))
            zero_outs.append(np.zeros(shape, dtype))
    n_params = len(in_names)
    all_in = list(in_names) + list(out_names)
    if partition_name is not None:
        all_in.append(partition_name)

    def _body(*args):
        operands = list(args)
        if partition_name is not None:
            operands.append(partition_id_tensor())
        outs = _bass_exec_p.bind(
            *operands,
            out_avals=tuple(out_avals),
            in_names=tuple(all_in),
            out_names=tuple(out_names),
            lowering_input_output_aliases=(),
            sim_require_finite=False,
            sim_require_nnan=False,
            nc=nc,
        )
        return tuple(outs)

    devices = jax.devices()[:NC]
    mesh = Mesh(np.asarray(devices), ("core",))
    in_specs = (PartitionSpec("core"),) * (n_params + len(out_names))
    out_specs = (PartitionSpec("core"),) * len(out_names)
    fn = jax.jit(
        shard_map(_body, mesh=mesh, in_specs=in_specs, out_specs=out_specs,
                  check_rep=False),
        keep_unused=True,
    )
    concat_in = [
        np.concatenate([np.asarray(in_maps[c][k]) for c in range(NC)], axis=0)
        for k in in_names
    ]
    concat_zero = [np.zeros((NC * z.shape[0], *z.shape[1:]), z.dtype) for z in zero_outs]
    sharding = NamedSharding(mesh, PartitionSpec("core"))
    dev_in = [jax.device_put(a, sharding) for a in concat_in + concat_zero]
    outs = fn(*dev_in)
    jax.block_until_ready(outs)
    res = [
        {name: np.asarray(outs[i]).reshape(NC, *out_avals[i].shape)[c]
         for i, name in enumerate(out_names)}
        for c in range(NC)
    ]
    return res, (fn, dev_in, out_names, out_avals)


def _make_in_maps(inputs, N, SH, SHP, NB, LAYERS, OUT):
    import ml_dtypes
    bf = ml_dtypes.bfloat16
    nodes = np.asarray(inputs["nodes"], np.float32)
    src = np.asarray(inputs["src"])
    dst = np.asarray(inputs["dst"])
    W_in = np.asarray(inputs["W_in"], np.float32)
    b_in = np.asarray(inputs["b_in"], np.float32)
    Ws = np.asarray(inputs["Ws"], np.float32)
    bs = np.asarray(inputs["bs"], np.float32)
    gammas = np.asarray(inputs["gammas"], np.float32)
    betas = np.asarray(inputs["betas"], np.float32)
    W_out = np.asarray(inputs["W_out"], np.float32)
    b_out = np.asarray(inputs["b_out"], np.float32)

    idx_dev, dst_dev, meta = _prep_edges(src, dst, N, SH, SHP, NB)

    deg = np.bincount(dst, minlength=N).astype(np.float32)
    inv = 1.0 / (deg + 1.0)
    invp = np.ones((NC, SHP), np.float32)
    invp.reshape(NC, SHP)[:, :SH] = inv.reshape(NC, SH)
    deg1 = np.ones((NC, SHP), np.float32)
    deg1.reshape(NC, SHP)[:, :SH] = (deg + 1.0).reshape(NC, SH)
    assert deg1.max() <= 256, "deg+1 must be bf16-exact"
    invp = np.ascontiguousarray(invp.reshape(NC, NB, P).transpose(0, 2, 1))

    nvalid = SH - (NB - 1) * P
    maskp = (np.arange(P) < nvalid).astype(np.float32)[:, None]

    Wl = np.zeros((LAYERS, P, P), np.float32)
    bl = np.zeros((LAYERS, P), np.float32)
    Wl[0] = Ws[0]
    bl[0] = bs[0]
    for l in range(1, LAYERS):
        Wl[l] = gammas[l - 1][:, None] * Ws[l]
        bl[l] = betas[l - 1] @ Ws[l] + bs[l]
    Wout = gammas[LAYERS - 1][:, None] * W_out
    bout = betas[LAYERS - 1] @ W_out + b_out

    iota = np.tile(np.arange(P, dtype=np.float32), (P, 1))

    in_maps = []
    for c in range(NC):
        nsh = np.zeros((SHP, P), bf)
        nsh[:SH] = nodes[c * SH:(c + 1) * SH].astype(bf)
        in_maps.append({
            "nodes_bf": np.ascontiguousarray(nsh.T),
            "idxs": idx_dev[c],
            "dstrel": dst_dev[c],
            "invp": invp[c],
            "deg1p": deg1[c][None, :].astype(bf),
            "maskp": maskp,
            "W_in": W_in.astype(bf),
            "Wl": Wl.astype(bf),
            "Wout": Wout.astype(bf),
            "b_in_r": b_in[None, :].astype(bf),
            "bl_r": bl[:, None, :].astype(bf),
            "bout_r": bout[None, :].astype(bf),
            "iota": iota.astype(bf),
        })
    return in_maps, meta


def kernel(**inputs):
    nodes = np.asarray(inputs["nodes"])
    N = nodes.shape[0]
    LAYERS = np.asarray(inputs["Ws"]).shape[0]
    OUT = np.asarray(inputs["W_out"]).shape[1]
    assert N % NC == 0
    SH = N // NC
    SHP = (SH + P - 1) // P * P
    NB = SHP // P
    assert SHP <= 32767, "int16 gather index limit"
    assert NB % SB == 0

    in_maps, meta = _make_in_maps(inputs, N, SH, SHP, NB, LAYERS, OUT)

    import hashlib
    h = hashlib.sha1(meta["caps"].tobytes()).hexdigest()[:12]
    key = (NB, SHP, LAYERS, OUT, h)
    if key not in _CACHE:
        _CACHE[key] = _build_program(NB, SHP, LAYERS, OUT, meta)
    nc_prog = _CACHE[key]

    res, exec_info = _run(nc_prog, in_maps)
    global _LAST_EXEC
    _LAST_EXEC = exec_info
    out = np.concatenate([res[c]["out_own"][:SH] for c in range(NC)], axis=0)
    return out.astype(np.float32)



# revision 29
# speedup vs baseline: 1.0190x; 1.0190x over previous
"""GNN message-passing (GCN-mean) kernel for 8 Trainium2 NeuronCores. (V6)

V3 (diagonal slotting) plus:
- K0=2 with overflow edges pooled per (superblock, group) into statically
  ranged cross-block chunks (90% slot fill vs 76%), cutting gather
  descriptors ~15%.
- pad slots spread across all SHP-SH zero rows (same-row descriptors
  measured ~7x slower on HBM).
- overflow edges src-sorted within each pool for HBM locality.
- one-hot masks built in ONE batched DVE op per block (broadcast APs),
  emitted one block ahead so PE never stalls on mask builds.
- aggregation accumulated TRANSPOSED (paggT[feat, dst] via lhsT=x chunks,
  rhs=identity/one-hot), which removes the per-block PE transpose + ACT
  copy; bias applied as outer(deg+1, b) so the 1/(deg+1) scale folds into
  the ReLU's per-partition scale; nodes supplied pre-transposed by host.
- deeper PSUM pools (3 bufs for agg/h banks).
"""
import math
import os
import numpy as np

NC = 8
P = 128
EPS = 1e-5
SB = 7        # dst blocks per superblock
NCHUNK = 7
K0 = 2        # diagonal chunks per (block, group)

_CACHE = {}
_LAST_EXEC = None


# --------------------------------------------------------------------------
# device program
# --------------------------------------------------------------------------
def _build_program(NB, SHP, LAYERS, OUT, meta):
    import concourse.bacc as bacc
    import concourse.mybir as mybir
    import concourse.tile as tile
    from concourse.masks import make_identity

    NSUP = NB // SB
    SUPC = NSUP // NCHUNK
    BLKC = NB // NCHUNK
    XCOLS = meta["XCOLS"]
    DCOLS = meta["DCOLS"]
    IDXC = meta["IDXC"]
    OHMAX = meta["OHMAX"]
    per_sg = meta["per_sg"]          # [NSUP, 8] chunks per gather
    g_off = meta["g_off"]            # [NSUP, 8]
    reg_off = meta["reg_off"]        # [NSUP, SB, 8] diag col base
    pairs = meta["pairs"]            # [s][bl] -> [(g, xg col, dst col), ...]
    f32 = mybir.dt.float32
    bf16 = mybir.dt.bfloat16
    Alu = mybir.AluOpType
    Act = mybir.ActivationFunctionType

    nc = bacc.Bacc("TRN2", target_bir_lowering=False, num_devices=NC,
                   num_swdge_queues=4)

    nodes_bf = nc.declare_dram_parameter("nodes_bf", [P, SHP], bf16, isOutput=False)
    idxs = nc.declare_dram_parameter("idxs", [NSUP, P, IDXC], mybir.dt.int16, isOutput=False)
    dstrel = nc.declare_dram_parameter("dstrel", [NSUP, P, DCOLS], bf16, isOutput=False)
    invp = nc.declare_dram_parameter("invp", [P, NB], f32, isOutput=False)
    deg1p = nc.declare_dram_parameter("deg1p", [1, NB * P], bf16, isOutput=False)
    maskp = nc.declare_dram_parameter("maskp", [P, 1], f32, isOutput=False)
    W_in = nc.declare_dram_parameter("W_in", [P, P], bf16, isOutput=False)
    Wl = nc.declare_dram_parameter("Wl", [LAYERS, P, P], bf16, isOutput=False)
    Wout = nc.declare_dram_parameter("Wout", [P, OUT], bf16, isOutput=False)
    b_in_r = nc.declare_dram_parameter("b_in_r", [1, P], bf16, isOutput=False)
    bl_r = nc.declare_dram_parameter("bl_r", [LAYERS, 1, P], bf16, isOutput=False)
    bout_r = nc.declare_dram_parameter("bout_r", [1, OUT], bf16, isOutput=False)
    iota_in = nc.declare_dram_parameter("iota", [P, P], bf16, isOutput=False)
    out_own = nc.declare_dram_parameter("out_own", [SHP, OUT], f32, isOutput=True)

    rg = [list(range(NC))]

    with tile.TileContext(nc) as tc:
        with (
            tc.tile_pool(name="const", bufs=1) as cpool,
            tc.tile_pool(name="dram", bufs=1, space="DRAM") as dpool,
            tc.tile_pool(name="work", bufs=6) as wpool,
            tc.tile_pool(name="ohp", bufs=4) as ohpool,
            tc.tile_pool(name="idxp", bufs=3) as ipool,
            tc.tile_pool(name="xgp", bufs=2) as xgpool,
            tc.tile_pool(name="psum", bufs=3, space="PSUM") as ppool,
            tc.tile_pool(name="psum1", bufs=1, space="PSUM") as ppool1,
        ):
            iota_t = cpool.tile([P, P], bf16)
            nc.sync.dma_start(out=iota_t[:], in_=iota_in[:])
            ident_bf = cpool.tile([P, P], bf16)
            make_identity(nc, ident_bf[:])
            eps_t = cpool.tile([P, 1], f32)
            nc.vector.memset(eps_t[:], EPS)
            W_in_t = cpool.tile([P, P], bf16)
            nc.sync.dma_start(out=W_in_t[:], in_=W_in[:])
            Wout_t = cpool.tile([P, OUT], bf16)
            nc.sync.dma_start(out=Wout_t[:], in_=Wout[:])
            ones1 = cpool.tile([1, P], bf16)
            nc.vector.memset(ones1[:], 1.0)
            bin_t = cpool.tile([1, P], bf16)
            nc.sync.dma_start(out=bin_t[:], in_=b_in_r[:])
            bl_t = []
            for l in range(LAYERS):
                t = cpool.tile([1, P], bf16, name=f"bl{l}")
                nc.sync.dma_start(out=t[:], in_=bl_r[l])
                bl_t.append(t)
            Wl_ts = []
            for l in range(LAYERS):
                t = cpool.tile([P, P], bf16, name=f"wl{l}")
                nc.sync.dma_start(out=t[:], in_=Wl[l])
                Wl_ts.append(t)
            bout_t = cpool.tile([1, OUT], bf16)
            nc.sync.dma_start(out=bout_t[:], in_=bout_r[:])
            inv_t = cpool.tile([P, NB], f32)
            nc.sync.dma_start(out=inv_t[:], in_=invp[:])
            mask_t = cpool.tile([P, 1], f32)
            nc.sync.dma_start(out=mask_t[:], in_=maskp[:])

            ag_in = [dpool.tile([SHP, P], bf16, name=f"ag_in{l}") for l in range(LAYERS)]
            x_full = [
                dpool.tile([NC, SHP, P], bf16, addr_space="Shared", name=f"x_full{l}")
                for l in range(LAYERS)
            ]

            def emit_ag(l_t, k):
                nc.gpsimd.collective_compute(
                    "AllGather", Alu.bypass, replica_groups=rg,
                    ins=[ag_in[l_t][:].opt()],
                    outs=[x_full[l_t][:].opt()],
                )

            # ---------------- Phase A: x0 = nodes @ W_in + b_in ----------
            for b in range(NB):
                nbT = wpool.tile([P, P], bf16, tag="nb")
                nc.sync.dma_start(out=nbT[:], in_=nodes_bf[:, b * P:(b + 1) * P])
                ph = ppool.tile([P, P], f32, tag="h", space="PSUM")
                nc.tensor.matmul(ph[:], lhsT=nbT[:], rhs=W_in_t[:], start=True, stop=False)
                nc.tensor.matmul(ph[:], lhsT=ones1[:], rhs=bin_t[:], start=False, stop=True)
                x0b = wpool.tile([P, P], bf16, tag="xnext")
                if b == NB - 1:
                    # zero the pad node rows so the gather zero-row stays 0
                    nc.scalar.activation(x0b[:], ph[:], Act.Copy, scale=mask_t[:, 0:1])
                else:
                    nc.scalar.copy(out=x0b[:], in_=ph[:])
                nc.sync.dma_start(out=ag_in[0][b * P:(b + 1) * P, :], in_=x0b[:])
            emit_ag(0, 0)

            # ---------------- Layers ------------------------------------
            for l in range(LAYERS):
                xf = x_full[l]
                last = l == LAYERS - 1
                for s in range(NSUP):
                    idx_t = ipool.tile([P, IDXC], mybir.dt.int16, tag="idx")
                    nc.sync.dma_start(out=idx_t[:], in_=idxs[s])
                    dst_t = ipool.tile([P, DCOLS], bf16, tag="dst")
                    nc.sync.dma_start(out=dst_t[:], in_=dstrel[s])
                    deg1_t = ipool.tile([1, SB * P], bf16, tag="deg1")
                    nc.sync.dma_start(out=deg1_t[:],
                                      in_=deg1p[0:1, s * SB * P:(s + 1) * SB * P])
                    xg = xgpool.tile([P, XCOLS, P], bf16, tag="xg")
                    for g in range(8):
                        nci = int(per_sg[s, g]) * P
                        co = int(g_off[s, g])
                        nc.gpsimd.dma_gather(
                            out_ap=xg[:, co:co + int(per_sg[s, g]), :],
                            in_ap=xf[g],
                            idxs_ap=idx_t[:, co * 8:co * 8 + nci // 16],
                            num_idxs=nci,
                            num_idxs_reg=nci,
                            elem_size=P,
                            single_packet=False,
                            queue_num=g % 4,
                        )
                    def build_oh(bl):
                        # one batched DVE op builds all of a block's one-hots
                        prs_b = pairs[s][bl]
                        t = ohpool.tile([P, OHMAX, P], bf16, tag="oh")
                        if prs_b:
                            noh_b = len(prs_b)
                            dc0 = prs_b[0][2]
                            nc.vector.tensor_tensor(
                                out=t[:, 0:noh_b, :],
                                in0=iota_t[:].unsqueeze(1).to_broadcast([P, noh_b, P]),
                                in1=dst_t[:, dc0:dc0 + noh_b].unsqueeze(2)
                                    .to_broadcast([P, noh_b, P]),
                                op=Alu.is_equal,
                            )
                        return t

                    oh_next = build_oh(0)
                    for bl_i in range(SB):
                        b = s * SB + bl_i
                        prs = pairs[s][bl_i]   # [(g, xg col, dst col), ...]
                        oh = oh_next
                        if bl_i + 1 < SB:
                            oh_next = build_oh(bl_i + 1)
                        xs_bf = wpool.tile([P, P], bf16, tag="xs")
                        nc.sync.dma_start(out=xs_bf[:], in_=ag_in[l][b * P:(b + 1) * P, :])
                        noh = len(prs)
                        # aggregate TRANSPOSED: paggT[feat, dst] avoids the
                        # per-block PE transpose + ACT copy before the W matmul
                        paggT = ppool.tile([P, P], f32, tag="agg", space="PSUM")
                        nmm = 1 + 8 * K0 + noh
                        nc.tensor.matmul(paggT[:], lhsT=xs_bf[:], rhs=ident_bf[:],
                                         start=True, stop=(nmm == 1))
                        mm = 1
                        for g in range(8):
                            ro = int(reg_off[s, bl_i, g])
                            for k in range(K0):
                                mm += 1
                                nc.tensor.matmul(
                                    paggT[:], lhsT=xg[:, ro + k, :], rhs=ident_bf[:],
                                    start=False, stop=(mm == nmm),
                                )
                        for j, (g_, col_, dc_) in enumerate(prs):
                            mm += 1
                            nc.tensor.matmul(
                                paggT[:], lhsT=xg[:, col_, :], rhs=oh[:, j, :],
                                start=False, stop=(mm == nmm),
                            )
                        # ---- fused epilogue ----
                        m1T = wpool.tile([P, P], bf16, tag="m1")
                        nc.scalar.copy(out=m1T[:], in_=paggT[:])
                        ph = ppool.tile([P, P], f32, tag="h", space="PSUM")
                        nc.tensor.matmul(ph[:], lhsT=m1T[:], rhs=Wl_ts[l][:],
                                         start=True, stop=False)
                        # bias as outer((deg+1), b): inv scale below folds it to +b
                        nc.tensor.matmul(ph[:], lhsT=deg1_t[0:1, bl_i * P:(bl_i + 1) * P],
                                         rhs=bl_t[l][:], start=False, stop=True)
                        hr = wpool.tile([P, P], f32, tag="hr")
                        mu_s = wpool.tile([P, 1], f32, tag="mus")
                        nc.scalar.activation(hr[:], ph[:], Act.Relu,
                                             scale=inv_t[:, b:b + 1], accum_out=mu_s[:])
                        h2 = wpool.tile([P, P], f32, tag="h2")
                        s2 = wpool.tile([P, 1], f32, tag="s2")
                        nc.scalar.activation(h2[:], hr[:], Act.Square, accum_out=s2[:])
                        musq = wpool.tile([P, 1], f32, tag="musq")
                        nc.scalar.activation(musq[:], mu_s[:], Act.Square,
                                             scale=1.0 / P)
                        var2 = wpool.tile([P, 1], f32, tag="var2")
                        nc.vector.scalar_tensor_tensor(
                            out=var2[:], in0=s2[:], scalar=1.0 / P, in1=musq[:],
                            op0=Alu.mult, op1=Alu.subtract,
                        )
                        std_t = wpool.tile([P, 1], f32, tag="std")
                        nc.scalar.activation(std_t[:], var2[:], Act.Sqrt, bias=eps_t[:, 0:1])
                        rstd = wpool.tile([P, 1], f32, tag="rstd")
                        nc.vector.reciprocal_approx_fast(out=rstd[:], in_=std_t[:])
                        mu_t = wpool.tile([P, 1], f32, tag="mu")
                        nc.scalar.activation(mu_t[:], mu_s[:], Act.Copy,
                                             scale=1.0 / P)
                        y_bf = wpool.tile([P, P], bf16, tag="xnext" if not last else "yf")
                        nc.vector.tensor_scalar(
                            out=y_bf[:], in0=hr[:], scalar1=mu_t[:, 0:1],
                            scalar2=rstd[:, 0:1], op0=Alu.subtract, op1=Alu.mult,
                        )
                        if not last:
                            if b == NB - 1:
                                ym = wpool.tile([P, P], bf16, tag="ym")
                                nc.vector.tensor_scalar(
                                    out=ym[:], in0=y_bf[:], scalar1=mask_t[:, 0:1],
                                    scalar2=None, op0=Alu.mult,
                                )
                                nc.sync.dma_start(out=ag_in[l + 1][b * P:(b + 1) * P, :], in_=ym[:])
                            else:
                                nc.sync.dma_start(out=ag_in[l + 1][b * P:(b + 1) * P, :], in_=y_bf[:])
                        else:
                            pyT = ppool1.tile([P, P], bf16, tag="mT", space="PSUM")
                            nc.tensor.transpose(pyT[:], y_bf[:], ident_bf[:])
                            yT = wpool.tile([P, P], bf16, tag="mTs")
                            nc.scalar.copy(out=yT[:], in_=pyT[:])
                            po = ppool1.tile([P, OUT], f32, tag="po", space="PSUM")
                            nc.tensor.matmul(po[:], lhsT=yT[:], rhs=Wout_t[:], start=True, stop=False)
                            nc.tensor.matmul(po[:], lhsT=ones1[:], rhs=bout_t[:], start=False, stop=True)
                            ob = wpool.tile([P, OUT], f32, tag="ob")
                            nc.scalar.copy(out=ob[:], in_=po[:])
                            nc.sync.dma_start(out=out_own[b * P:(b + 1) * P, :], in_=ob[:])
                if not last:
                    emit_ag(l + 1, 0)

    nc.compile()
    return nc


# --------------------------------------------------------------------------
# host-side sharding prep (diagonal slotting)
# --------------------------------------------------------------------------
def _prep_edges(src, dst, N, SH, SHP, NB):
    E = src.shape[0]
    src = src.astype(np.int64)
    dst = dst.astype(np.int64)
    NSUP = NB // SB
    ZR = SHP - 1  # zero row within each group table

    core = dst // SH
    dst_loc = dst - core * SH
    blk = dst_loc >> 7
    dst_rel = dst_loc & 127
    grp = src // SH
    src_loc = src - grp * SH

    key = ((core * NB + blk) * 8 + grp) * P + dst_rel
    order = np.lexsort((src_loc, key))
    ks = key[order]
    sl = src_loc[order]
    dr = dst_rel[order]

    cell_counts = np.bincount(ks, minlength=NC * NB * 8 * P)
    cell_starts = np.zeros_like(cell_counts)
    np.cumsum(cell_counts[:-1], out=cell_starts[1:])
    rank = np.arange(E, dtype=np.int64) - cell_starts[ks]

    cbg = ks // P
    d = ks % P

    is_diag = rank < K0
    ovf_mask = ~is_diag

    # overflow edges pooled per (superblock, group) with STATIC per-block
    # ranges (max over cores) so chunk->block structure is core-independent
    ovf_counts = np.bincount(cbg[ovf_mask], minlength=NC * NB * 8).reshape(NC, NB, 8)
    R_sb = ovf_counts.max(axis=0).reshape(NSUP, SB, 8)   # static range [s, bl, g]
    start_off = np.zeros((NSUP, SB, 8), np.int64)
    start_off[:, 1:, :] = np.cumsum(R_sb, axis=1)[:, :-1, :]
    pool = R_sb.sum(axis=1)                              # [s, g]
    novf_chunks = -(-pool // P)                          # [s, g]

    per_sg = SB * K0 + novf_chunks                       # [s, g] cols per gather
    g_off = np.zeros((NSUP, 8), np.int64)
    g_off[:, 1:] = np.cumsum(per_sg[:, :-1], axis=1)
    reg_off = (g_off[:, None, :] +
               (np.arange(SB) * K0)[None, :, None])      # diag col base [s, bl, g]
    ovf_base = g_off + SB * K0                           # first ovf col [s, g]
    XCOLS = int(per_sg.sum(axis=1).max())

    # (chunk, block) pair lists per (s, bl): static from ranges
    pairs = [[[] for _ in range(SB)] for _ in range(NSUP)]
    maxci = int(novf_chunks.max())
    dcol_lut = np.full((NSUP, 8, maxci, SB), -1, np.int64)
    DCOLS = 0
    for s in range(NSUP):
        dcol = 0
        for bl in range(SB):
            for gg in range(8):
                rn = int(R_sb[s, bl, gg])
                if rn == 0:
                    continue
                st = int(start_off[s, bl, gg])
                for ci in range(st >> 7, ((st + rn - 1) >> 7) + 1):
                    pairs[s][bl].append((gg, int(ovf_base[s, gg] + ci), dcol))
                    dcol_lut[s, gg, ci, bl] = dcol
                    dcol += 1
        DCOLS = max(DCOLS, dcol)
    OHMAX = max(len(pairs[s][bl]) for s in range(NSUP) for bl in range(SB))

    sup_e = (cbg // 8) % NB // SB
    b_local = ((cbg // 8) % NB) % SB
    g = cbg % 8
    c_ = cbg // (NB * 8)

    ovf_rank = np.zeros(E, np.int64)
    oc = np.bincount(cbg[ovf_mask], minlength=NC * NB * 8)
    os_ = np.zeros_like(oc)
    np.cumsum(oc[:-1], out=os_[1:])
    # rank overflow edges within their (core, block, group) pool in
    # src-ascending order for HBM gather locality
    ovf_idx = np.nonzero(ovf_mask)[0]
    o2 = np.lexsort((sl[ovf_idx], cbg[ovf_idx]))
    sorted_pos = ovf_idx[o2]
    ovf_rank[sorted_pos] = np.arange(len(ovf_idx)) - os_[cbg[sorted_pos]]

    pool_pos = start_off[sup_e, b_local, g] + ovf_rank
    chunk = np.where(
        is_diag,
        reg_off[sup_e, b_local, g] + rank,
        ovf_base[sup_e, g] + (pool_pos >> 7),
    )
    part = np.where(is_diag, d, pool_pos & 127)

    IDX_TOT = XCOLS * P
    # pad slots gather zero rows; spread across all SHP-SH zero rows to avoid
    # HBM same-address hotspotting (same-row descriptors measured ~7x slower)
    NZ = SHP - SH
    padpat = (SH + (np.arange(IDX_TOT) % NZ)).astype(np.int16)
    idx16 = np.broadcast_to(padpat, (NC, NSUP, IDX_TOT)).copy()
    slot = chunk * P + part
    idx16[c_, sup_e, slot] = sl.astype(np.int16)

    dstv = np.full((NC, NSUP, DCOLS, P), -1.0, np.float32)
    m = ovf_mask
    ocol = dcol_lut[sup_e[m], g[m], pool_pos[m] >> 7, b_local[m]]
    assert (ocol >= 0).all()
    dstv[c_[m], sup_e[m], ocol, part[m]] = dr[m].astype(np.float32)
    import ml_dtypes
    dst_dev = np.ascontiguousarray(dstv.transpose(0, 1, 3, 2)).astype(ml_dtypes.bfloat16)

    IDXC = IDX_TOT // 16
    A = idx16.reshape(NC, NSUP, IDXC, 16)
    Bm = A.transpose(0, 1, 3, 2)
    idx_dev = np.ascontiguousarray(
        np.broadcast_to(Bm[:, :, None, :, :], (NC, NSUP, 8, 16, IDXC))
        .reshape(NC, NSUP, P, IDXC)
    )
    meta = dict(caps=R_sb, per_sg=per_sg, g_off=g_off, reg_off=reg_off,
                pairs=pairs, XCOLS=XCOLS,
                DCOLS=DCOLS, IDXC=IDXC, OHMAX=OHMAX)
    return idx_dev, dst_dev, meta


def _run(nc_prog, in_maps):
    import jax
    import numpy as np
    from jax.sharding import Mesh, PartitionSpec, NamedSharding
    from jax.experimental.shard_map import shard_map
    import concourse.mybir as mybir
    from concourse.bass2jax import _bass_exec_p, install_neuronx_cc_hook, partition_id_tensor

    install_neuronx_cc_hook()
    nc = nc_prog
    partition_name = nc.partition_id_tensor.name if nc.partition_id_tensor else None
    in_names, out_names, out_avals, zero_outs = [], [], [], []
    for alloc in nc.m.functions[0].allocations:
        if not isinstance(alloc, mybir.MemoryLocationSet):
            continue
        name = alloc.memorylocations[0].name
        if alloc.kind == "ExternalInput":
            if name != partition_name:
                in_names.append(name)
        elif alloc.kind == "ExternalOutput":
            out_names.append(name)
            shape = tuple(alloc.tensor_shape)
            dtype = mybir.dt.np(alloc.dtype)
            out_avals.append(jax.core.ShapedArray(shape, dtype))
            zero_outs.append(np.zeros(shape, dtype))
    n_params = len(in_names)
    all_in = list(in_names) + list(out_names)
    if partition_name is not None:
        all_in.append(partition_name)

    def _body(*args):
        operands = list(args)
        if partition_name is not None:
            operands.append(partition_id_tensor())
        outs = _bass_exec_p.bind(
            *operands,
            out_avals=tuple(out_avals),
            in_names=tuple(all_in),
            out_names=tuple(out_names),
            lowering_input_output_aliases=(),
            sim_require_finite=False,
            sim_require_nnan=False,
            nc=nc,
        )
        return tuple(outs)

    devices = jax.devices()[:NC]
    mesh = Mesh(np.asarray(devices), ("core",))
    in_specs = (PartitionSpec("core"),) * (n_params + len(out_names))
    out_specs = (PartitionSpec("core"),) * len(out_names)
    fn = jax.jit(
        shard_map(_body, mesh=mesh, in_specs=in_specs, out_specs=out_specs,
                  check_rep=False),
        keep_unused=True,
    )
    concat_in = [
        np.concatenate([np.asarray(in_maps[c][k]) for c in range(NC)], axis=0)
        for k in in_names
    ]
    concat_zero = [np.zeros((NC * z.shape[0], *z.shape[1:]), z.dtype) for z in zero_outs]
    sharding = NamedSharding(mesh, PartitionSpec("core"))
    dev_in = [jax.device_put(a, sharding) for a in concat_in + concat_zero]
    outs = fn(*dev_in)
    jax.block_until_ready(outs)
    res = [
        {name: np.asarray(outs[i]).reshape(NC, *out_avals[i].shape)[c]
         for i, name in enumerate(out_names)}
        for c in range(NC)
    ]
    return res, (fn, dev_in, out_names, out_avals)


def _make_in_maps(inputs, N, SH, SHP, NB, LAYERS, OUT):
    import ml_dtypes
    bf = ml_dtypes.bfloat16
    nodes = np.asarray(inputs["nodes"], np.float32)
    src = np.asarray(inputs["src"])
    dst = np.asarray(inputs["dst"])
    W_in = np.asarray(inputs["W_in"], np.float32)
    b_in = np.asarray(inputs["b_in"], np.float32)
    Ws = np.asarray(inputs["Ws"], np.float32)
    bs = np.asarray(inputs["bs"], np.float32)
    gammas = np.asarray(inputs["gammas"], np.float32)
    betas = np.asarray(inputs["betas"], np.float32)
    W_out = np.asarray(inputs["W_out"], np.float32)
    b_out = np.asarray(inputs["b_out"], np.float32)

    idx_dev, dst_dev, meta = _prep_edges(src, dst, N, SH, SHP, NB)

    deg = np.bincount(dst, minlength=N).astype(np.float32)
    inv = 1.0 / (deg + 1.0)
    invp = np.ones((NC, SHP), np.float32)
    invp.reshape(NC, SHP)[:, :SH] = inv.reshape(NC, SH)
    deg1 = np.ones((NC, SHP), np.float32)
    deg1.reshape(NC, SHP)[:, :SH] = (deg + 1.0).reshape(NC, SH)
    assert deg1.max() <= 256, "deg+1 must be bf16-exact"
    invp = np.ascontiguousarray(invp.reshape(NC, NB, P).transpose(0, 2, 1))

    nvalid = SH - (NB - 1) * P
    maskp = (np.arange(P) < nvalid).astype(np.float32)[:, None]

    Wl = np.zeros((LAYERS, P, P), np.float32)
    bl = np.zeros((LAYERS, P), np.float32)
    Wl[0] = Ws[0]
    bl[0] = bs[0]
    for l in range(1, LAYERS):
        Wl[l] = gammas[l - 1][:, None] * Ws[l]
        bl[l] = betas[l - 1] @ Ws[l] + bs[l]
    Wout = gammas[LAYERS - 1][:, None] * W_out
    bout = betas[LAYERS - 1] @ W_out + b_out

    iota = np.tile(np.arange(P, dtype=np.float32), (P, 1))

    in_maps = []
    for c in range(NC):
        nsh = np.zeros((SHP, P), bf)
        nsh[:SH] = nodes[c * SH:(c + 1) * SH].astype(bf)
        in_maps.append({
            "nodes_bf": np.ascontiguousarray(nsh.T),
            "idxs": idx_dev[c],
            "dstrel": dst_dev[c],
            "invp": invp[c],
            "deg1p": deg1[c][None, :].astype(bf),
            "maskp": maskp,
            "W_in": W_in.astype(bf),
            "Wl": Wl.astype(bf),
            "Wout": Wout.astype(bf),
            "b_in_r": b_in[None, :].astype(bf),
            "bl_r": bl[:, None, :].astype(bf),
            "bout_r": bout[None, :].astype(bf),
            "iota": iota.astype(bf),
        })
    return in_maps, meta


def kernel(**inputs):
    nodes = np.asarray(inputs["nodes"])
    N = nodes.shape[0]
    LAYERS = np.asarray(inputs["Ws"]).shape[0]
    OUT = np.asarray(inputs["W_out"]).shape[1]
    assert N % NC == 0
    SH = N // NC
    SHP = (SH + P - 1) // P * P
    NB = SHP // P
    assert SHP <= 32767, "int16 gather index limit"
    assert NB % SB == 0

    in_maps, meta = _make_in_maps(inputs, N, SH, SHP, NB, LAYERS, OUT)

    import hashlib
    h = hashlib.sha1(meta["caps"].tobytes()).hexdigest()[:12]
    key = (NB, SHP, LAYERS, OUT, h)
    if key not in _CACHE:
        _CACHE[key] = _build_program(NB, SHP, LAYERS, OUT, meta)
    nc_prog = _CACHE[key]

    res, exec_info = _run(nc_prog, in_maps)
    global _LAST_EXEC
    _LAST_EXEC = exec_info
    out = np.concatenate([res[c]["out_own"][:SH] for c in range(NC)], axis=0)
    return out.astype(np.float32)



# revision 30
# speedup vs baseline: 1.0437x; 1.0243x over previous
"""GNN message-passing (GCN-mean) kernel for 8 Trainium2 NeuronCores. (V6)

V3 (diagonal slotting) plus:
- K0=2 with overflow edges pooled per (superblock, group) into statically
  ranged cross-block chunks (90% slot fill vs 76%), cutting gather
  descriptors ~15%.
- pad slots spread across all SHP-SH zero rows (same-row descriptors
  measured ~7x slower on HBM).
- overflow edges src-sorted within each pool for HBM locality.
- one-hot masks built in ONE batched DVE op per block (broadcast APs),
  emitted one block ahead so PE never stalls on mask builds.
- aggregation accumulated TRANSPOSED (paggT[feat, dst] via lhsT=x chunks,
  rhs=identity/one-hot), which removes the per-block PE transpose + ACT
  copy; bias applied as outer(deg+1, b) so the 1/(deg+1) scale folds into
  the ReLU's per-partition scale; nodes supplied pre-transposed by host.
- deeper PSUM pools (3 bufs for agg/h banks).
"""
import math
import os
import numpy as np

NC = 8
P = 128
EPS = 1e-5
SB = 7        # dst blocks per superblock
NCHUNK = 7
K0 = 2        # diagonal chunks per (block, group)

_CACHE = {}
_LAST_EXEC = None


# --------------------------------------------------------------------------
# device program
# --------------------------------------------------------------------------
def _build_program(NB, SHP, LAYERS, OUT, meta):
    import concourse.bacc as bacc
    import concourse.mybir as mybir
    import concourse.tile as tile
    from concourse.masks import make_identity

    NSUP = NB // SB
    SUPC = NSUP // NCHUNK
    BLKC = NB // NCHUNK
    XCOLS = meta["XCOLS"]
    DCOLS = meta["DCOLS"]
    IDXC = meta["IDXC"]
    OHMAX = meta["OHMAX"]
    per_sg = meta["per_sg"]          # [NSUP, 8] chunks per gather
    g_off = meta["g_off"]            # [NSUP, 8]
    reg_off = meta["reg_off"]        # [NSUP, SB, 8] diag col base
    pairs = meta["pairs"]            # [s][bl] -> [(g, xg col, dst col), ...]
    f32 = mybir.dt.float32
    bf16 = mybir.dt.bfloat16
    Alu = mybir.AluOpType
    Act = mybir.ActivationFunctionType

    nc = bacc.Bacc("TRN2", target_bir_lowering=False, num_devices=NC,
                   num_swdge_queues=4)

    nodes_bf = nc.declare_dram_parameter("nodes_bf", [P, SHP], bf16, isOutput=False)
    idxs = nc.declare_dram_parameter("idxs", [NSUP, P, IDXC], mybir.dt.int16, isOutput=False)
    dstrel = nc.declare_dram_parameter("dstrel", [NSUP, P, DCOLS], bf16, isOutput=False)
    invp = nc.declare_dram_parameter("invp", [P, NB], f32, isOutput=False)
    deg1p = nc.declare_dram_parameter("deg1p", [1, NB * P], bf16, isOutput=False)
    maskp = nc.declare_dram_parameter("maskp", [P, 1], f32, isOutput=False)
    W_in = nc.declare_dram_parameter("W_in", [P, P], bf16, isOutput=False)
    Wl = nc.declare_dram_parameter("Wl", [LAYERS, P, P], bf16, isOutput=False)
    Wout = nc.declare_dram_parameter("Wout", [P, OUT], bf16, isOutput=False)
    b_in_r = nc.declare_dram_parameter("b_in_r", [1, P], bf16, isOutput=False)
    bl_r = nc.declare_dram_parameter("bl_r", [LAYERS, 1, P], bf16, isOutput=False)
    bout_r = nc.declare_dram_parameter("bout_r", [1, OUT], bf16, isOutput=False)
    iota_in = nc.declare_dram_parameter("iota", [P, P], bf16, isOutput=False)
    out_own = nc.declare_dram_parameter("out_own", [SHP, OUT], f32, isOutput=True)

    rg = [list(range(NC))]

    with tile.TileContext(nc) as tc:
        with (
            tc.tile_pool(name="const", bufs=1) as cpool,
            tc.tile_pool(name="dram", bufs=1, space="DRAM") as dpool,
            tc.tile_pool(name="work", bufs=6) as wpool,
            tc.tile_pool(name="ohp", bufs=5) as ohpool,
            tc.tile_pool(name="idxp", bufs=3) as ipool,
            tc.tile_pool(name="xgp", bufs=2) as xgpool,
            tc.tile_pool(name="psum", bufs=3, space="PSUM") as ppool,
            tc.tile_pool(name="psum1", bufs=1, space="PSUM") as ppool1,
        ):
            iota_t = cpool.tile([P, P], bf16)
            nc.sync.dma_start(out=iota_t[:], in_=iota_in[:])
            ident_bf = cpool.tile([P, P], bf16)
            make_identity(nc, ident_bf[:])
            eps_t = cpool.tile([P, 1], f32)
            nc.vector.memset(eps_t[:], EPS)
            W_in_t = cpool.tile([P, P], bf16)
            nc.sync.dma_start(out=W_in_t[:], in_=W_in[:])
            Wout_t = cpool.tile([P, OUT], bf16)
            nc.sync.dma_start(out=Wout_t[:], in_=Wout[:])
            ones1 = cpool.tile([1, P], bf16)
            nc.vector.memset(ones1[:], 1.0)
            bin_t = cpool.tile([1, P], bf16)
            nc.sync.dma_start(out=bin_t[:], in_=b_in_r[:])
            bl_t = []
            for l in range(LAYERS):
                t = cpool.tile([1, P], bf16, name=f"bl{l}")
                nc.sync.dma_start(out=t[:], in_=bl_r[l])
                bl_t.append(t)
            Wl_ts = []
            for l in range(LAYERS):
                t = cpool.tile([P, P], bf16, name=f"wl{l}")
                nc.sync.dma_start(out=t[:], in_=Wl[l])
                Wl_ts.append(t)
            bout_t = cpool.tile([1, OUT], bf16)
            nc.sync.dma_start(out=bout_t[:], in_=bout_r[:])
            inv_t = cpool.tile([P, NB], f32)
            nc.sync.dma_start(out=inv_t[:], in_=invp[:])
            mask_t = cpool.tile([P, 1], f32)
            nc.sync.dma_start(out=mask_t[:], in_=maskp[:])

            ag_in = [dpool.tile([SHP, P], bf16, name=f"ag_in{l}") for l in range(LAYERS)]
            x_full = [
                dpool.tile([NC, SHP, P], bf16, addr_space="Shared", name=f"x_full{l}")
                for l in range(LAYERS)
            ]

            def emit_ag(l_t, k):
                nc.gpsimd.collective_compute(
                    "AllGather", Alu.bypass, replica_groups=rg,
                    ins=[ag_in[l_t][:].opt()],
                    outs=[x_full[l_t][:].opt()],
                )

            # ---------------- Phase A: x0 = nodes @ W_in + b_in ----------
            for b in range(NB):
                nbT = wpool.tile([P, P], bf16, tag="nb")
                nc.sync.dma_start(out=nbT[:], in_=nodes_bf[:, b * P:(b + 1) * P])
                ph = ppool.tile([P, P], f32, tag="h", space="PSUM")
                nc.tensor.matmul(ph[:], lhsT=nbT[:], rhs=W_in_t[:], start=True, stop=False)
                nc.tensor.matmul(ph[:], lhsT=ones1[:], rhs=bin_t[:], start=False, stop=True)
                x0b = wpool.tile([P, P], bf16, tag="xnext")
                if b == NB - 1:
                    # zero the pad node rows so the gather zero-row stays 0
                    nc.scalar.activation(x0b[:], ph[:], Act.Copy, scale=mask_t[:, 0:1])
                else:
                    nc.scalar.copy(out=x0b[:], in_=ph[:])
                nc.sync.dma_start(out=ag_in[0][b * P:(b + 1) * P, :], in_=x0b[:])
            emit_ag(0, 0)

            # ---------------- Layers ------------------------------------
            for l in range(LAYERS):
                xf = x_full[l]
                last = l == LAYERS - 1
                for s in range(NSUP):
                    idx_t = ipool.tile([P, IDXC], mybir.dt.int16, tag="idx")
                    nc.sync.dma_start(out=idx_t[:], in_=idxs[s])
                    dst_t = ipool.tile([P, DCOLS], bf16, tag="dst")
                    nc.sync.dma_start(out=dst_t[:], in_=dstrel[s])
                    deg1_t = ipool.tile([1, SB * P], bf16, tag="deg1")
                    nc.sync.dma_start(out=deg1_t[:],
                                      in_=deg1p[0:1, s * SB * P:(s + 1) * SB * P])
                    xg = xgpool.tile([P, XCOLS, P], bf16, tag="xg")
                    for g in range(8):
                        nci = int(per_sg[s, g]) * P
                        co = int(g_off[s, g])
                        nc.gpsimd.dma_gather(
                            out_ap=xg[:, co:co + int(per_sg[s, g]), :],
                            in_ap=xf[g],
                            idxs_ap=idx_t[:, co * 8:co * 8 + nci // 16],
                            num_idxs=nci,
                            num_idxs_reg=nci,
                            elem_size=P,
                            single_packet=False,
                            queue_num=g % 4,
                        )
                    def build_oh(bl):
                        # one batched DVE op builds all of a block's one-hots
                        prs_b = pairs[s][bl]
                        t = ohpool.tile([P, OHMAX, P], bf16, tag="oh")
                        if prs_b:
                            noh_b = len(prs_b)
                            dc0 = prs_b[0][2]
                            nc.vector.tensor_tensor(
                                out=t[:, 0:noh_b, :],
                                in0=iota_t[:].unsqueeze(1).to_broadcast([P, noh_b, P]),
                                in1=dst_t[:, dc0:dc0 + noh_b].unsqueeze(2)
                                    .to_broadcast([P, noh_b, P]),
                                op=Alu.is_equal,
                            )
                        return t

                    oh_q = [build_oh(0), build_oh(1)]
                    for bl_i in range(SB):
                        b = s * SB + bl_i
                        prs = pairs[s][bl_i]   # [(g, xg col, dst col), ...]
                        oh = oh_q.pop(0)
                        if bl_i + 2 < SB:
                            oh_q.append(build_oh(bl_i + 2))
                        xs_bf = wpool.tile([P, P], bf16, tag="xs")
                        nc.sync.dma_start(out=xs_bf[:], in_=ag_in[l][b * P:(b + 1) * P, :])
                        noh = len(prs)
                        # aggregate TRANSPOSED: paggT[feat, dst] avoids the
                        # per-block PE transpose + ACT copy before the W matmul
                        paggT = ppool.tile([P, P], f32, tag="agg", space="PSUM")
                        nmm = 1 + 8 * K0 + noh
                        nc.tensor.matmul(paggT[:], lhsT=xs_bf[:], rhs=ident_bf[:],
                                         start=True, stop=(nmm == 1))
                        mm = 1
                        for g in range(8):
                            ro = int(reg_off[s, bl_i, g])
                            for k in range(K0):
                                mm += 1
                                nc.tensor.matmul(
                                    paggT[:], lhsT=xg[:, ro + k, :], rhs=ident_bf[:],
                                    start=False, stop=(mm == nmm),
                                )
                        for j, (g_, col_, dc_) in enumerate(prs):
                            mm += 1
                            nc.tensor.matmul(
                                paggT[:], lhsT=xg[:, col_, :], rhs=oh[:, j, :],
                                start=False, stop=(mm == nmm),
                            )
                        # ---- fused epilogue ----
                        m1T = wpool.tile([P, P], bf16, tag="m1")
                        nc.scalar.copy(out=m1T[:], in_=paggT[:])
                        ph = ppool.tile([P, P], f32, tag="h", space="PSUM")
                        nc.tensor.matmul(ph[:], lhsT=m1T[:], rhs=Wl_ts[l][:],
                                         start=True, stop=False)
                        # bias as outer((deg+1), b): inv scale below folds it to +b
                        nc.tensor.matmul(ph[:], lhsT=deg1_t[0:1, bl_i * P:(bl_i + 1) * P],
                                         rhs=bl_t[l][:], start=False, stop=True)
                        hr = wpool.tile([P, P], f32, tag="hr")
                        mu_s = wpool.tile([P, 1], f32, tag="mus")
                        nc.scalar.activation(hr[:], ph[:], Act.Relu,
                                             scale=inv_t[:, b:b + 1], accum_out=mu_s[:])
                        h2 = wpool.tile([P, P], f32, tag="h2")
                        s2 = wpool.tile([P, 1], f32, tag="s2")
                        nc.scalar.activation(h2[:], hr[:], Act.Square, accum_out=s2[:])
                        musq = wpool.tile([P, 1], f32, tag="musq")
                        nc.scalar.activation(musq[:], mu_s[:], Act.Square,
                                             scale=1.0 / P)
                        var2 = wpool.tile([P, 1], f32, tag="var2")
                        nc.vector.scalar_tensor_tensor(
                            out=var2[:], in0=s2[:], scalar=1.0 / P, in1=musq[:],
                            op0=Alu.mult, op1=Alu.subtract,
                        )
                        std_t = wpool.tile([P, 1], f32, tag="std")
                        nc.scalar.activation(std_t[:], var2[:], Act.Sqrt, bias=eps_t[:, 0:1])
                        rstd = wpool.tile([P, 1], f32, tag="rstd")
                        nc.vector.reciprocal_approx_fast(out=rstd[:], in_=std_t[:])
                        mu_t = wpool.tile([P, 1], f32, tag="mu")
                        nc.scalar.activation(mu_t[:], mu_s[:], Act.Copy,
                                             scale=1.0 / P)
                        y_bf = wpool.tile([P, P], bf16, tag="xnext" if not last else "yf")
                        nc.vector.tensor_scalar(
                            out=y_bf[:], in0=hr[:], scalar1=mu_t[:, 0:1],
                            scalar2=rstd[:, 0:1], op0=Alu.subtract, op1=Alu.mult,
                        )
                        if not last:
                            if b == NB - 1:
                                ym = wpool.tile([P, P], bf16, tag="ym")
                                nc.vector.tensor_scalar(
                                    out=ym[:], in0=y_bf[:], scalar1=mask_t[:, 0:1],
                                    scalar2=None, op0=Alu.mult,
                                )
                                nc.sync.dma_start(out=ag_in[l + 1][b * P:(b + 1) * P, :], in_=ym[:])
                            else:
                                nc.sync.dma_start(out=ag_in[l + 1][b * P:(b + 1) * P, :], in_=y_bf[:])
                        else:
                            pyT = ppool1.tile([P, P], bf16, tag="mT", space="PSUM")
                            nc.tensor.transpose(pyT[:], y_bf[:], ident_bf[:])
                            yT = wpool.tile([P, P], bf16, tag="mTs")
                            nc.scalar.copy(out=yT[:], in_=pyT[:])
                            po = ppool1.tile([P, OUT], f32, tag="po", space="PSUM")
                            nc.tensor.matmul(po[:], lhsT=yT[:], rhs=Wout_t[:], start=True, stop=False)
                            nc.tensor.matmul(po[:], lhsT=ones1[:], rhs=bout_t[:], start=False, stop=True)
                            ob = wpool.tile([P, OUT], f32, tag="ob")
                            nc.scalar.copy(out=ob[:], in_=po[:])
                            nc.sync.dma_start(out=out_own[b * P:(b + 1) * P, :], in_=ob[:])
                if not last:
                    emit_ag(l + 1, 0)

    nc.compile()
    return nc


# --------------------------------------------------------------------------
# host-side sharding prep (diagonal slotting)
# --------------------------------------------------------------------------
def _prep_edges(src, dst, N, SH, SHP, NB):
    E = src.shape[0]
    src = src.astype(np.int64)
    dst = dst.astype(np.int64)
    NSUP = NB // SB
    ZR = SHP - 1  # zero row within each group table

    core = dst // SH
    dst_loc = dst - core * SH
    blk = dst_loc >> 7
    dst_rel = dst_loc & 127
    grp = src // SH
    src_loc = src - grp * SH

    key = ((core * NB + blk) * 8 + grp) * P + dst_rel
    order = np.lexsort((src_loc, key))
    ks = key[order]
    sl = src_loc[order]
    dr = dst_rel[order]

    cell_counts = np.bincount(ks, minlength=NC * NB * 8 * P)
    cell_starts = np.zeros_like(cell_counts)
    np.cumsum(cell_counts[:-1], out=cell_starts[1:])
    rank = np.arange(E, dtype=np.int64) - cell_starts[ks]

    cbg = ks // P
    d = ks % P

    is_diag = rank < K0
    ovf_mask = ~is_diag

    # overflow edges pooled per (superblock, group) with STATIC per-block
    # ranges (max over cores) so chunk->block structure is core-independent
    ovf_counts = np.bincount(cbg[ovf_mask], minlength=NC * NB * 8).reshape(NC, NB, 8)
    R_sb = ovf_counts.max(axis=0).reshape(NSUP, SB, 8)   # static range [s, bl, g]
    start_off = np.zeros((NSUP, SB, 8), np.int64)
    start_off[:, 1:, :] = np.cumsum(R_sb, axis=1)[:, :-1, :]
    pool = R_sb.sum(axis=1)                              # [s, g]
    novf_chunks = -(-pool // P)                          # [s, g]

    per_sg = SB * K0 + novf_chunks                       # [s, g] cols per gather
    g_off = np.zeros((NSUP, 8), np.int64)
    g_off[:, 1:] = np.cumsum(per_sg[:, :-1], axis=1)
    reg_off = (g_off[:, None, :] +
               (np.arange(SB) * K0)[None, :, None])      # diag col base [s, bl, g]
    ovf_base = g_off + SB * K0                           # first ovf col [s, g]
    XCOLS = int(per_sg.sum(axis=1).max())

    # (chunk, block) pair lists per (s, bl): static from ranges
    pairs = [[[] for _ in range(SB)] for _ in range(NSUP)]
    maxci = int(novf_chunks.max())
    dcol_lut = np.full((NSUP, 8, maxci, SB), -1, np.int64)
    DCOLS = 0
    for s in range(NSUP):
        dcol = 0
        for bl in range(SB):
            for gg in range(8):
                rn = int(R_sb[s, bl, gg])
                if rn == 0:
                    continue
                st = int(start_off[s, bl, gg])
                for ci in range(st >> 7, ((st + rn - 1) >> 7) + 1):
                    pairs[s][bl].append((gg, int(ovf_base[s, gg] + ci), dcol))
                    dcol_lut[s, gg, ci, bl] = dcol
                    dcol += 1
        DCOLS = max(DCOLS, dcol)
    OHMAX = max(len(pairs[s][bl]) for s in range(NSUP) for bl in range(SB))

    sup_e = (cbg // 8) % NB // SB
    b_local = ((cbg // 8) % NB) % SB
    g = cbg % 8
    c_ = cbg // (NB * 8)

    ovf_rank = np.zeros(E, np.int64)
    oc = np.bincount(cbg[ovf_mask], minlength=NC * NB * 8)
    os_ = np.zeros_like(oc)
    np.cumsum(oc[:-1], out=os_[1:])
    # rank overflow edges within their (core, block, group) pool in
    # src-ascending order for HBM gather locality
    ovf_idx = np.nonzero(ovf_mask)[0]
    o2 = np.lexsort((sl[ovf_idx], cbg[ovf_idx]))
    sorted_pos = ovf_idx[o2]
    ovf_rank[sorted_pos] = np.arange(len(ovf_idx)) - os_[cbg[sorted_pos]]

    pool_pos = start_off[sup_e, b_local, g] + ovf_rank
    chunk = np.where(
        is_diag,
        reg_off[sup_e, b_local, g] + rank,
        ovf_base[sup_e, g] + (pool_pos >> 7),
    )
    part = np.where(is_diag, d, pool_pos & 127)

    IDX_TOT = XCOLS * P
    # pad slots gather zero rows; spread across all SHP-SH zero rows to avoid
    # HBM same-address hotspotting (same-row descriptors measured ~7x slower)
    NZ = SHP - SH
    padpat = (SH + (np.arange(IDX_TOT) % NZ)).astype(np.int16)
    idx16 = np.broadcast_to(padpat, (NC, NSUP, IDX_TOT)).copy()
    slot = chunk * P + part
    idx16[c_, sup_e, slot] = sl.astype(np.int16)

    dstv = np.full((NC, NSUP, DCOLS, P), -1.0, np.float32)
    m = ovf_mask
    ocol = dcol_lut[sup_e[m], g[m], pool_pos[m] >> 7, b_local[m]]
    assert (ocol >= 0).all()
    dstv[c_[m], sup_e[m], ocol, part[m]] = dr[m].astype(np.float32)
    import ml_dtypes
    dst_dev = np.ascontiguousarray(dstv.transpose(0, 1, 3, 2)).astype(ml_dtypes.bfloat16)

    IDXC = IDX_TOT // 16
    A = idx16.reshape(NC, NSUP, IDXC, 16)
    Bm = A.transpose(0, 1, 3, 2)
    idx_dev = np.ascontiguousarray(
        np.broadcast_to(Bm[:, :, None, :, :], (NC, NSUP, 8, 16, IDXC))
        .reshape(NC, NSUP, P, IDXC)
    )
    meta = dict(caps=R_sb, per_sg=per_sg, g_off=g_off, reg_off=reg_off,
                pairs=pairs, XCOLS=XCOLS,
                DCOLS=DCOLS, IDXC=IDXC, OHMAX=OHMAX)
    return idx_dev, dst_dev, meta


def _run(nc_prog, in_maps):
    import jax
    import numpy as np
    from jax.sharding import Mesh, PartitionSpec, NamedSharding
    from jax.experimental.shard_map import shard_map
    import concourse.mybir as mybir
    from concourse.bass2jax import _bass_exec_p, install_neuronx_cc_hook, partition_id_tensor

    install_neuronx_cc_hook()
    nc = nc_prog
    partition_name = nc.partition_id_tensor.name if nc.partition_id_tensor else None
    in_names, out_names, out_avals, zero_outs = [], [], [], []
    for alloc in nc.m.functions[0].allocations:
        if not isinstance(alloc, mybir.MemoryLocationSet):
            continue
        name = alloc.memorylocations[0].name
        if alloc.kind == "ExternalInput":
            if name != partition_name:
                in_names.append(name)
        elif alloc.kind == "ExternalOutput":
            out_names.append(name)
            shape = tuple(alloc.tensor_shape)
            dtype = mybir.dt.np(alloc.dtype)
            out_avals.append(jax.core.ShapedArray(shape, dtype))
            zero_outs.append(np.zeros(shape, dtype))
    n_params = len(in_names)
    all_in = list(in_names) + list(out_names)
    if partition_name is not None:
        all_in.append(partition_name)

    def _body(*args):
        operands = list(args)
        if partition_name is not None:
            operands.append(partition_id_tensor())
        outs = _bass_exec_p.bind(
            *operands,
            out_avals=tuple(out_avals),
            in_names=tuple(all_in),
            out_names=tuple(out_names),
            lowering_input_output_aliases=(),
            sim_require_finite=False,
            sim_require_nnan=False,
            nc=nc,
        )
        return tuple(outs)

    devices = jax.devices()[:NC]
    mesh = Mesh(np.asarray(devices), ("core",))
    in_specs = (PartitionSpec("core"),) * (n_params + len(out_names))
    out_specs = (PartitionSpec("core"),) * len(out_names)
    fn = jax.jit(
        shard_map(_body, mesh=mesh, in_specs=in_specs, out_specs=out_specs,
                  check_rep=False),
        keep_unused=True,
    )
    concat_in = [
        np.concatenate([np.asarray(in_maps[c][k]) for c in range(NC)], axis=0)
        for k in in_names
    ]
    concat_zero = [np.zeros((NC * z.shape[0], *z.shape[1:]), z.dtype) for z in zero_outs]
    sharding = NamedSharding(mesh, PartitionSpec("core"))
    dev_in = [jax.device_put(a, sharding) for a in concat_in + concat_zero]
    outs = fn(*dev_in)
    jax.block_until_ready(outs)
    res = [
        {name: np.asarray(outs[i]).reshape(NC, *out_avals[i].shape)[c]
         for i, name in enumerate(out_names)}
        for c in range(NC)
    ]
    return res, (fn, dev_in, out_names, out_avals)


def _make_in_maps(inputs, N, SH, SHP, NB, LAYERS, OUT):
    import ml_dtypes
    bf = ml_dtypes.bfloat16
    nodes = np.asarray(inputs["nodes"], np.float32)
    src = np.asarray(inputs["src"])
    dst = np.asarray(inputs["dst"])
    W_in = np.asarray(inputs["W_in"], np.float32)
    b_in = np.asarray(inputs["b_in"], np.float32)
    Ws = np.asarray(inputs["Ws"], np.float32)
    bs = np.asarray(inputs["bs"], np.float32)
    gammas = np.asarray(inputs["gammas"], np.float32)
    betas = np.asarray(inputs["betas"], np.float32)
    W_out = np.asarray(inputs["W_out"], np.float32)
    b_out = np.asarray(inputs["b_out"], np.float32)

    idx_dev, dst_dev, meta = _prep_edges(src, dst, N, SH, SHP, NB)

    deg = np.bincount(dst, minlength=N).astype(np.float32)
    inv = 1.0 / (deg + 1.0)
    invp = np.ones((NC, SHP), np.float32)
    invp.reshape(NC, SHP)[:, :SH] = inv.reshape(NC, SH)
    deg1 = np.ones((NC, SHP), np.float32)
    deg1.reshape(NC, SHP)[:, :SH] = (deg + 1.0).reshape(NC, SH)
    assert deg1.max() <= 256, "deg+1 must be bf16-exact"
    invp = np.ascontiguousarray(invp.reshape(NC, NB, P).transpose(0, 2, 1))

    nvalid = SH - (NB - 1) * P
    maskp = (np.arange(P) < nvalid).astype(np.float32)[:, None]

    Wl = np.zeros((LAYERS, P, P), np.float32)
    bl = np.zeros((LAYERS, P), np.float32)
    Wl[0] = Ws[0]
    bl[0] = bs[0]
    for l in range(1, LAYERS):
        Wl[l] = gammas[l - 1][:, None] * Ws[l]
        bl[l] = betas[l - 1] @ Ws[l] + bs[l]
    Wout = gammas[LAYERS - 1][:, None] * W_out
    bout = betas[LAYERS - 1] @ W_out + b_out

    iota = np.tile(np.arange(P, dtype=np.float32), (P, 1))

    in_maps = []
    for c in range(NC):
        nsh = np.zeros((SHP, P), bf)
        nsh[:SH] = nodes[c * SH:(c + 1) * SH].astype(bf)
        in_maps.append({
            "nodes_bf": np.ascontiguousarray(nsh.T),
            "idxs": idx_dev[c],
            "dstrel": dst_dev[c],
            "invp": invp[c],
            "deg1p": deg1[c][None, :].astype(bf),
            "maskp": maskp,
            "W_in": W_in.astype(bf),
            "Wl": Wl.astype(bf),
            "Wout": Wout.astype(bf),
            "b_in_r": b_in[None, :].astype(bf),
            "bl_r": bl[:, None, :].astype(bf),
            "bout_r": bout[None, :].astype(bf),
            "iota": iota.astype(bf),
        })
    return in_maps, meta


def kernel(**inputs):
    nodes = np.asarray(inputs["nodes"])
    N = nodes.shape[0]
    LAYERS = np.asarray(inputs["Ws"]).shape[0]
    OUT = np.asarray(inputs["W_out"]).shape[1]
    assert N % NC == 0
    SH = N // NC
    SHP = (SH + P - 1) // P * P
    NB = SHP // P
    assert SHP <= 32767, "int16 gather index limit"
    assert NB % SB == 0

    in_maps, meta = _make_in_maps(inputs, N, SH, SHP, NB, LAYERS, OUT)

    import hashlib
    h = hashlib.sha1(meta["caps"].tobytes()).hexdigest()[:12]
    key = (NB, SHP, LAYERS, OUT, h)
    if key not in _CACHE:
        _CACHE[key] = _build_program(NB, SHP, LAYERS, OUT, meta)
    nc_prog = _CACHE[key]

    res, exec_info = _run(nc_prog, in_maps)
    global _LAST_EXEC
    _LAST_EXEC = exec_info
    out = np.concatenate([res[c]["out_own"][:SH] for c in range(NC)], axis=0)
    return out.astype(np.float32)



# revision 31
# speedup vs baseline: 1.0879x; 1.0423x over previous
"""GNN message-passing (GCN-mean) kernel for 8 Trainium2 NeuronCores. (V6)

V3 (diagonal slotting) plus:
- K0=2 with overflow edges pooled per (superblock, group) into statically
  ranged cross-block chunks (90% slot fill vs 76%), cutting gather
  descriptors ~15%.
- pad slots spread across all SHP-SH zero rows (same-row descriptors
  measured ~7x slower on HBM).
- overflow edges src-sorted within each pool for HBM locality.
- one-hot masks built in ONE batched DVE op per block (broadcast APs),
  emitted one block ahead so PE never stalls on mask builds.
- aggregation accumulated TRANSPOSED (paggT[feat, dst] via lhsT=x chunks,
  rhs=identity/one-hot), which removes the per-block PE transpose + ACT
  copy; bias applied as outer(deg+1, b) so the 1/(deg+1) scale folds into
  the ReLU's per-partition scale; nodes supplied pre-transposed by host.
- deeper PSUM pools (3 bufs for agg/h banks).
"""
import math
import os
import numpy as np

NC = 8
P = 128
EPS = 1e-5
SB = 7        # dst blocks per superblock
NCHUNK = 7
K0 = 2        # diagonal chunks per (block, group)

_CACHE = {}
_LAST_EXEC = None


# --------------------------------------------------------------------------
# device program
# --------------------------------------------------------------------------
def _build_program(NB, SHP, LAYERS, OUT, meta):
    import concourse.bacc as bacc
    import concourse.mybir as mybir
    import concourse.tile as tile
    from concourse.masks import make_identity

    NSUP = NB // SB
    SUPC = NSUP // NCHUNK
    BLKC = NB // NCHUNK
    XCOLS = meta["XCOLS"]
    DCOLS = meta["DCOLS"]
    IDXC = meta["IDXC"]
    OHMAX = meta["OHMAX"]
    per_sg = meta["per_sg"]          # [NSUP, 8] chunks per gather
    g_off = meta["g_off"]            # [NSUP, 8]
    reg_off = meta["reg_off"]        # [NSUP, SB, 8] diag col base
    pairs = meta["pairs"]            # [s][bl] -> [(g, xg col, dst col), ...]
    f32 = mybir.dt.float32
    bf16 = mybir.dt.bfloat16
    Alu = mybir.AluOpType
    Act = mybir.ActivationFunctionType

    nc = bacc.Bacc("TRN2", target_bir_lowering=False, num_devices=NC,
                   num_swdge_queues=4)

    nodes_bf = nc.declare_dram_parameter("nodes_bf", [P, SHP], bf16, isOutput=False)
    idxs = nc.declare_dram_parameter("idxs", [NSUP, P, IDXC], mybir.dt.int16, isOutput=False)
    dstrel = nc.declare_dram_parameter("dstrel", [NSUP, P, DCOLS], bf16, isOutput=False)
    invp = nc.declare_dram_parameter("invp", [P, NB], f32, isOutput=False)
    deg1p = nc.declare_dram_parameter("deg1p", [1, NB * P], bf16, isOutput=False)
    maskp = nc.declare_dram_parameter("maskp", [P, 1], f32, isOutput=False)
    W_in = nc.declare_dram_parameter("W_in", [P, P], bf16, isOutput=False)
    Wl = nc.declare_dram_parameter("Wl", [LAYERS, P, P], bf16, isOutput=False)
    Wout = nc.declare_dram_parameter("Wout", [P, OUT], bf16, isOutput=False)
    b_in_r = nc.declare_dram_parameter("b_in_r", [1, P], bf16, isOutput=False)
    bl_r = nc.declare_dram_parameter("bl_r", [LAYERS, 1, P], bf16, isOutput=False)
    bout_r = nc.declare_dram_parameter("bout_r", [1, OUT], bf16, isOutput=False)
    iota_in = nc.declare_dram_parameter("iota", [P, P], bf16, isOutput=False)
    out_own = nc.declare_dram_parameter("out_own", [SHP, OUT], f32, isOutput=True)

    rg = [list(range(NC))]

    with tile.TileContext(nc) as tc:
        with (
            tc.tile_pool(name="const", bufs=1) as cpool,
            tc.tile_pool(name="dram", bufs=1, space="DRAM") as dpool,
            tc.tile_pool(name="work", bufs=6) as wpool,
            tc.tile_pool(name="ohp", bufs=5) as ohpool,
            tc.tile_pool(name="idxp", bufs=3) as ipool,
            tc.tile_pool(name="xgp", bufs=2) as xgpool,
            tc.tile_pool(name="psum", bufs=4, space="PSUM") as ppool,
            tc.tile_pool(name="psumh", bufs=2, space="PSUM") as ppoolh,
            tc.tile_pool(name="psum1", bufs=1, space="PSUM") as ppool1,
        ):
            iota_t = cpool.tile([P, P], bf16)
            nc.sync.dma_start(out=iota_t[:], in_=iota_in[:])
            ident_bf = cpool.tile([P, P], bf16)
            make_identity(nc, ident_bf[:])
            eps_t = cpool.tile([P, 1], f32)
            nc.vector.memset(eps_t[:], EPS)
            W_in_t = cpool.tile([P, P], bf16)
            nc.sync.dma_start(out=W_in_t[:], in_=W_in[:])
            Wout_t = cpool.tile([P, OUT], bf16)
            nc.sync.dma_start(out=Wout_t[:], in_=Wout[:])
            ones1 = cpool.tile([1, P], bf16)
            nc.vector.memset(ones1[:], 1.0)
            bin_t = cpool.tile([1, P], bf16)
            nc.sync.dma_start(out=bin_t[:], in_=b_in_r[:])
            bl_t = []
            for l in range(LAYERS):
                t = cpool.tile([1, P], bf16, name=f"bl{l}")
                nc.sync.dma_start(out=t[:], in_=bl_r[l])
                bl_t.append(t)
            Wl_ts = []
            for l in range(LAYERS):
                t = cpool.tile([P, P], bf16, name=f"wl{l}")
                nc.sync.dma_start(out=t[:], in_=Wl[l])
                Wl_ts.append(t)
            bout_t = cpool.tile([1, OUT], bf16)
            nc.sync.dma_start(out=bout_t[:], in_=bout_r[:])
            inv_t = cpool.tile([P, NB], f32)
            nc.sync.dma_start(out=inv_t[:], in_=invp[:])
            mask_t = cpool.tile([P, 1], f32)
            nc.sync.dma_start(out=mask_t[:], in_=maskp[:])

            ag_in = [dpool.tile([SHP, P], bf16, name=f"ag_in{l}") for l in range(LAYERS)]
            x_full = [
                dpool.tile([NC, SHP, P], bf16, addr_space="Shared", name=f"x_full{l}")
                for l in range(LAYERS)
            ]

            def emit_ag(l_t, k):
                nc.gpsimd.collective_compute(
                    "AllGather", Alu.bypass, replica_groups=rg,
                    ins=[ag_in[l_t][:].opt()],
                    outs=[x_full[l_t][:].opt()],
                )

            # ---------------- Phase A: x0 = nodes @ W_in + b_in ----------
            for b in range(NB):
                nbT = wpool.tile([P, P], bf16, tag="nb")
                nc.sync.dma_start(out=nbT[:], in_=nodes_bf[:, b * P:(b + 1) * P])
                ph = ppoolh.tile([P, P], f32, tag="h", space="PSUM")
                nc.tensor.matmul(ph[:], lhsT=nbT[:], rhs=W_in_t[:], start=True, stop=False)
                nc.tensor.matmul(ph[:], lhsT=ones1[:], rhs=bin_t[:], start=False, stop=True)
                x0b = wpool.tile([P, P], bf16, tag="xnext")
                if b == NB - 1:
                    # zero the pad node rows so the gather zero-row stays 0
                    nc.scalar.activation(x0b[:], ph[:], Act.Copy, scale=mask_t[:, 0:1])
                else:
                    nc.scalar.copy(out=x0b[:], in_=ph[:])
                nc.sync.dma_start(out=ag_in[0][b * P:(b + 1) * P, :], in_=x0b[:])
            emit_ag(0, 0)

            # ---------------- Layers ------------------------------------
            for l in range(LAYERS):
                xf = x_full[l]
                last = l == LAYERS - 1
                for s in range(NSUP):
                    idx_t = ipool.tile([P, IDXC], mybir.dt.int16, tag="idx")
                    nc.sync.dma_start(out=idx_t[:], in_=idxs[s])
                    dst_t = ipool.tile([P, DCOLS], bf16, tag="dst")
                    nc.sync.dma_start(out=dst_t[:], in_=dstrel[s])
                    deg1_t = ipool.tile([1, SB * P], bf16, tag="deg1")
                    nc.sync.dma_start(out=deg1_t[:],
                                      in_=deg1p[0:1, s * SB * P:(s + 1) * SB * P])
                    xg = xgpool.tile([P, XCOLS, P], bf16, tag="xg")
                    for g in range(8):
                        nci = int(per_sg[s, g]) * P
                        co = int(g_off[s, g])
                        nc.gpsimd.dma_gather(
                            out_ap=xg[:, co:co + int(per_sg[s, g]), :],
                            in_ap=xf[g],
                            idxs_ap=idx_t[:, co * 8:co * 8 + nci // 16],
                            num_idxs=nci,
                            num_idxs_reg=nci,
                            elem_size=P,
                            single_packet=False,
                            queue_num=g % 4,
                        )
                    def build_oh(bl):
                        # one batched DVE op builds all of a block's one-hots
                        prs_b = pairs[s][bl]
                        t = ohpool.tile([P, OHMAX, P], bf16, tag="oh")
                        if prs_b:
                            noh_b = len(prs_b)
                            dc0 = prs_b[0][2]
                            nc.vector.tensor_tensor(
                                out=t[:, 0:noh_b, :],
                                in0=iota_t[:].unsqueeze(1).to_broadcast([P, noh_b, P]),
                                in1=dst_t[:, dc0:dc0 + noh_b].unsqueeze(2)
                                    .to_broadcast([P, noh_b, P]),
                                op=Alu.is_equal,
                            )
                        return t

                    oh_q = [build_oh(0), build_oh(1)]
                    for bl_i in range(SB):
                        b = s * SB + bl_i
                        prs = pairs[s][bl_i]   # [(g, xg col, dst col), ...]
                        oh = oh_q.pop(0)
                        if bl_i + 2 < SB:
                            oh_q.append(build_oh(bl_i + 2))
                        xs_bf = wpool.tile([P, P], bf16, tag="xs")
                        nc.sync.dma_start(out=xs_bf[:], in_=ag_in[l][b * P:(b + 1) * P, :])
                        noh = len(prs)
                        # aggregate TRANSPOSED: paggT[feat, dst] avoids the
                        # per-block PE transpose + ACT copy before the W matmul
                        paggT = ppool.tile([P, P], f32, tag="agg", space="PSUM")
                        nmm = 1 + 8 * K0 + noh
                        nc.tensor.matmul(paggT[:], lhsT=xs_bf[:], rhs=ident_bf[:],
                                         start=True, stop=(nmm == 1))
                        mm = 1
                        for g in range(8):
                            ro = int(reg_off[s, bl_i, g])
                            for k in range(K0):
                                mm += 1
                                nc.tensor.matmul(
                                    paggT[:], lhsT=xg[:, ro + k, :], rhs=ident_bf[:],
                                    start=False, stop=(mm == nmm),
                                )
                        for j, (g_, col_, dc_) in enumerate(prs):
                            mm += 1
                            nc.tensor.matmul(
                                paggT[:], lhsT=xg[:, col_, :], rhs=oh[:, j, :],
                                start=False, stop=(mm == nmm),
                            )
                        # ---- fused epilogue ----
                        m1T = wpool.tile([P, P], bf16, tag="m1")
                        nc.scalar.copy(out=m1T[:], in_=paggT[:])
                        ph = ppoolh.tile([P, P], f32, tag="h", space="PSUM")
                        nc.tensor.matmul(ph[:], lhsT=m1T[:], rhs=Wl_ts[l][:],
                                         start=True, stop=False)
                        # bias as outer((deg+1), b): inv scale below folds it to +b
                        nc.tensor.matmul(ph[:], lhsT=deg1_t[0:1, bl_i * P:(bl_i + 1) * P],
                                         rhs=bl_t[l][:], start=False, stop=True)
                        hr = wpool.tile([P, P], f32, tag="hr")
                        mu_s = wpool.tile([P, 1], f32, tag="mus")
                        nc.scalar.activation(hr[:], ph[:], Act.Relu,
                                             scale=inv_t[:, b:b + 1], accum_out=mu_s[:])
                        h2 = wpool.tile([P, P], f32, tag="h2")
                        s2 = wpool.tile([P, 1], f32, tag="s2")
                        nc.scalar.activation(h2[:], hr[:], Act.Square, accum_out=s2[:])
                        musq = wpool.tile([P, 1], f32, tag="musq")
                        nc.scalar.activation(musq[:], mu_s[:], Act.Square,
                                             scale=1.0 / P)
                        var2 = wpool.tile([P, 1], f32, tag="var2")
                        nc.vector.scalar_tensor_tensor(
                            out=var2[:], in0=s2[:], scalar=1.0 / P, in1=musq[:],
                            op0=Alu.mult, op1=Alu.subtract,
                        )
                        std_t = wpool.tile([P, 1], f32, tag="std")
                        nc.scalar.activation(std_t[:], var2[:], Act.Sqrt, bias=eps_t[:, 0:1])
                        rstd = wpool.tile([P, 1], f32, tag="rstd")
                        nc.vector.reciprocal_approx_fast(out=rstd[:], in_=std_t[:])
                        mu_t = wpool.tile([P, 1], f32, tag="mu")
                        nc.scalar.activation(mu_t[:], mu_s[:], Act.Copy,
                                             scale=1.0 / P)
                        y_bf = wpool.tile([P, P], bf16, tag="xnext" if not last else "yf")
                        nc.vector.tensor_scalar(
                            out=y_bf[:], in0=hr[:], scalar1=mu_t[:, 0:1],
                            scalar2=rstd[:, 0:1], op0=Alu.subtract, op1=Alu.mult,
                        )
                        if not last:
                            if b == NB - 1:
                                ym = wpool.tile([P, P], bf16, tag="ym")
                                nc.vector.tensor_scalar(
                                    out=ym[:], in0=y_bf[:], scalar1=mask_t[:, 0:1],
                                    scalar2=None, op0=Alu.mult,
                                )
                                nc.sync.dma_start(out=ag_in[l + 1][b * P:(b + 1) * P, :], in_=ym[:])
                            else:
                                nc.sync.dma_start(out=ag_in[l + 1][b * P:(b + 1) * P, :], in_=y_bf[:])
                        else:
                            pyT = ppool1.tile([P, P], bf16, tag="mT", space="PSUM")
                            nc.tensor.transpose(pyT[:], y_bf[:], ident_bf[:])
                            yT = wpool.tile([P, P], bf16, tag="mTs")
                            nc.scalar.copy(out=yT[:], in_=pyT[:])
                            po = ppool1.tile([P, OUT], f32, tag="po", space="PSUM")
                            nc.tensor.matmul(po[:], lhsT=yT[:], rhs=Wout_t[:], start=True, stop=False)
                            nc.tensor.matmul(po[:], lhsT=ones1[:], rhs=bout_t[:], start=False, stop=True)
                            ob = wpool.tile([P, OUT], f32, tag="ob")
                            nc.scalar.copy(out=ob[:], in_=po[:])
                            nc.sync.dma_start(out=out_own[b * P:(b + 1) * P, :], in_=ob[:])
                if not last:
                    emit_ag(l + 1, 0)

    nc.compile()
    return nc


# --------------------------------------------------------------------------
# host-side sharding prep (diagonal slotting)
# --------------------------------------------------------------------------
def _prep_edges(src, dst, N, SH, SHP, NB):
    E = src.shape[0]
    src = src.astype(np.int64)
    dst = dst.astype(np.int64)
    NSUP = NB // SB
    ZR = SHP - 1  # zero row within each group table

    core = dst // SH
    dst_loc = dst - core * SH
    blk = dst_loc >> 7
    dst_rel = dst_loc & 127
    grp = src // SH
    src_loc = src - grp * SH

    key = ((core * NB + blk) * 8 + grp) * P + dst_rel
    order = np.lexsort((src_loc, key))
    ks = key[order]
    sl = src_loc[order]
    dr = dst_rel[order]

    cell_counts = np.bincount(ks, minlength=NC * NB * 8 * P)
    cell_starts = np.zeros_like(cell_counts)
    np.cumsum(cell_counts[:-1], out=cell_starts[1:])
    rank = np.arange(E, dtype=np.int64) - cell_starts[ks]

    cbg = ks // P
    d = ks % P

    is_diag = rank < K0
    ovf_mask = ~is_diag

    # overflow edges pooled per (superblock, group) with STATIC per-block
    # ranges (max over cores) so chunk->block structure is core-independent
    ovf_counts = np.bincount(cbg[ovf_mask], minlength=NC * NB * 8).reshape(NC, NB, 8)
    R_sb = ovf_counts.max(axis=0).reshape(NSUP, SB, 8)   # static range [s, bl, g]
    start_off = np.zeros((NSUP, SB, 8), np.int64)
    start_off[:, 1:, :] = np.cumsum(R_sb, axis=1)[:, :-1, :]
    pool = R_sb.sum(axis=1)                              # [s, g]
    novf_chunks = -(-pool // P)                          # [s, g]

    per_sg = SB * K0 + novf_chunks                       # [s, g] cols per gather
    g_off = np.zeros((NSUP, 8), np.int64)
    g_off[:, 1:] = np.cumsum(per_sg[:, :-1], axis=1)
    reg_off = (g_off[:, None, :] +
               (np.arange(SB) * K0)[None, :, None])      # diag col base [s, bl, g]
    ovf_base = g_off + SB * K0                           # first ovf col [s, g]
    XCOLS = int(per_sg.sum(axis=1).max())

    # (chunk, block) pair lists per (s, bl): static from ranges
    pairs = [[[] for _ in range(SB)] for _ in range(NSUP)]
    maxci = int(novf_chunks.max())
    dcol_lut = np.full((NSUP, 8, maxci, SB), -1, np.int64)
    DCOLS = 0
    for s in range(NSUP):
        dcol = 0
        for bl in range(SB):
            for gg in range(8):
                rn = int(R_sb[s, bl, gg])
                if rn == 0:
                    continue
                st = int(start_off[s, bl, gg])
                for ci in range(st >> 7, ((st + rn - 1) >> 7) + 1):
                    pairs[s][bl].append((gg, int(ovf_base[s, gg] + ci), dcol))
                    dcol_lut[s, gg, ci, bl] = dcol
                    dcol += 1
        DCOLS = max(DCOLS, dcol)
    OHMAX = max(len(pairs[s][bl]) for s in range(NSUP) for bl in range(SB))

    sup_e = (cbg // 8) % NB // SB
    b_local = ((cbg // 8) % NB) % SB
    g = cbg % 8
    c_ = cbg // (NB * 8)

    ovf_rank = np.zeros(E, np.int64)
    oc = np.bincount(cbg[ovf_mask], minlength=NC * NB * 8)
    os_ = np.zeros_like(oc)
    np.cumsum(oc[:-1], out=os_[1:])
    # rank overflow edges within their (core, block, group) pool in
    # src-ascending order for HBM gather locality
    ovf_idx = np.nonzero(ovf_mask)[0]
    o2 = np.lexsort((sl[ovf_idx], cbg[ovf_idx]))
    sorted_pos = ovf_idx[o2]
    ovf_rank[sorted_pos] = np.arange(len(ovf_idx)) - os_[cbg[sorted_pos]]

    pool_pos = start_off[sup_e, b_local, g] + ovf_rank
    chunk = np.where(
        is_diag,
        reg_off[sup_e, b_local, g] + rank,
        ovf_base[sup_e, g] + (pool_pos >> 7),
    )
    part = np.where(is_diag, d, pool_pos & 127)

    IDX_TOT = XCOLS * P
    # pad slots gather zero rows; spread across all SHP-SH zero rows to avoid
    # HBM same-address hotspotting (same-row descriptors measured ~7x slower)
    NZ = SHP - SH
    padpat = (SH + (np.arange(IDX_TOT) % NZ)).astype(np.int16)
    idx16 = np.broadcast_to(padpat, (NC, NSUP, IDX_TOT)).copy()
    slot = chunk * P + part
    idx16[c_, sup_e, slot] = sl.astype(np.int16)

    dstv = np.full((NC, NSUP, DCOLS, P), -1.0, np.float32)
    m = ovf_mask
    ocol = dcol_lut[sup_e[m], g[m], pool_pos[m] >> 7, b_local[m]]
    assert (ocol >= 0).all()
    dstv[c_[m], sup_e[m], ocol, part[m]] = dr[m].astype(np.float32)
    import ml_dtypes
    dst_dev = np.ascontiguousarray(dstv.transpose(0, 1, 3, 2)).astype(ml_dtypes.bfloat16)

    IDXC = IDX_TOT // 16
    A = idx16.reshape(NC, NSUP, IDXC, 16)
    Bm = A.transpose(0, 1, 3, 2)
    idx_dev = np.ascontiguousarray(
        np.broadcast_to(Bm[:, :, None, :, :], (NC, NSUP, 8, 16, IDXC))
        .reshape(NC, NSUP, P, IDXC)
    )
    meta = dict(caps=R_sb, per_sg=per_sg, g_off=g_off, reg_off=reg_off,
                pairs=pairs, XCOLS=XCOLS,
                DCOLS=DCOLS, IDXC=IDXC, OHMAX=OHMAX)
    return idx_dev, dst_dev, meta


def _run(nc_prog, in_maps):
    import jax
    import numpy as np
    from jax.sharding import Mesh, PartitionSpec, NamedSharding
    from jax.experimental.shard_map import shard_map
    import concourse.mybir as mybir
    from concourse.bass2jax import _bass_exec_p, install_neuronx_cc_hook, partition_id_tensor

    install_neuronx_cc_hook()
    nc = nc_prog
    partition_name = nc.partition_id_tensor.name if nc.partition_id_tensor else None
    in_names, out_names, out_avals, zero_outs = [], [], [], []
    for alloc in nc.m.functions[0].allocations:
        if not isinstance(alloc, mybir.MemoryLocationSet):
            continue
        name = alloc.memorylocations[0].name
        if alloc.kind == "ExternalInput":
            if name != partition_name:
                in_names.append(name)
        elif alloc.kind == "ExternalOutput":
            out_names.append(name)
            shape = tuple(alloc.tensor_shape)
            dtype = mybir.dt.np(alloc.dtype)
            out_avals.append(jax.core.ShapedArray(shape, dtype))
            zero_outs.append(np.zeros(shape, dtype))
    n_params = len(in_names)
    all_in = list(in_names) + list(out_names)
    if partition_name is not None:
        all_in.append(partition_name)

    def _body(*args):
        operands = list(args)
        if partition_name is not None:
            operands.append(partition_id_tensor())
        outs = _bass_exec_p.bind(
            *operands,
            out_avals=tuple(out_avals),
            in_names=tuple(all_in),
            out_names=tuple(out_names),
            lowering_input_output_aliases=(),
            sim_require_finite=False,
            sim_require_nnan=False,
            nc=nc,
        )
        return tuple(outs)

    devices = jax.devices()[:NC]
    mesh = Mesh(np.asarray(devices), ("core",))
    in_specs = (PartitionSpec("core"),) * (n_params + len(out_names))
    out_specs = (PartitionSpec("core"),) * len(out_names)
    fn = jax.jit(
        shard_map(_body, mesh=mesh, in_specs=in_specs, out_specs=out_specs,
                  check_rep=False),
        keep_unused=True,
    )
    concat_in = [
        np.concatenate([np.asarray(in_maps[c][k]) for c in range(NC)], axis=0)
        for k in in_names
    ]
    concat_zero = [np.zeros((NC * z.shape[0], *z.shape[1:]), z.dtype) for z in zero_outs]
    sharding = NamedSharding(mesh, PartitionSpec("core"))
    dev_in = [jax.device_put(a, sharding) for a in concat_in + concat_zero]
    outs = fn(*dev_in)
    jax.block_until_ready(outs)
    res = [
        {name: np.asarray(outs[i]).reshape(NC, *out_avals[i].shape)[c]
         for i, name in enumerate(out_names)}
        for c in range(NC)
    ]
    return res, (fn, dev_in, out_names, out_avals)


def _make_in_maps(inputs, N, SH, SHP, NB, LAYERS, OUT):
    import ml_dtypes
    bf = ml_dtypes.bfloat16
    nodes = np.asarray(inputs["nodes"], np.float32)
    src = np.asarray(inputs["src"])
    dst = np.asarray(inputs["dst"])
    W_in = np.asarray(inputs["W_in"], np.float32)
    b_in = np.asarray(inputs["b_in"], np.float32)
    Ws = np.asarray(inputs["Ws"], np.float32)
    bs = np.asarray(inputs["bs"], np.float32)
    gammas = np.asarray(inputs["gammas"], np.float32)
    betas = np.asarray(inputs["betas"], np.float32)
    W_out = np.asarray(inputs["W_out"], np.float32)
    b_out = np.asarray(inputs["b_out"], np.float32)

    idx_dev, dst_dev, meta = _prep_edges(src, dst, N, SH, SHP, NB)

    deg = np.bincount(dst, minlength=N).astype(np.float32)
    inv = 1.0 / (deg + 1.0)
    invp = np.ones((NC, SHP), np.float32)
    invp.reshape(NC, SHP)[:, :SH] = inv.reshape(NC, SH)
    deg1 = np.ones((NC, SHP), np.float32)
    deg1.reshape(NC, SHP)[:, :SH] = (deg + 1.0).reshape(NC, SH)
    assert deg1.max() <= 256, "deg+1 must be bf16-exact"
    invp = np.ascontiguousarray(invp.reshape(NC, NB, P).transpose(0, 2, 1))

    nvalid = SH - (NB - 1) * P
    maskp = (np.arange(P) < nvalid).astype(np.float32)[:, None]

    Wl = np.zeros((LAYERS, P, P), np.float32)
    bl = np.zeros((LAYERS, P), np.float32)
    Wl[0] = Ws[0]
    bl[0] = bs[0]
    for l in range(1, LAYERS):
        Wl[l] = gammas[l - 1][:, None] * Ws[l]
        bl[l] = betas[l - 1] @ Ws[l] + bs[l]
    Wout = gammas[LAYERS - 1][:, None] * W_out
    bout = betas[LAYERS - 1] @ W_out + b_out

    iota = np.tile(np.arange(P, dtype=np.float32), (P, 1))

    in_maps = []
    for c in range(NC):
        nsh = np.zeros((SHP, P), bf)
        nsh[:SH] = nodes[c * SH:(c + 1) * SH].astype(bf)
        in_maps.append({
            "nodes_bf": np.ascontiguousarray(nsh.T),
            "idxs": idx_dev[c],
            "dstrel": dst_dev[c],
            "invp": invp[c],
            "deg1p": deg1[c][None, :].astype(bf),
            "maskp": maskp,
            "W_in": W_in.astype(bf),
            "Wl": Wl.astype(bf),
            "Wout": Wout.astype(bf),
            "b_in_r": b_in[None, :].astype(bf),
            "bl_r": bl[:, None, :].astype(bf),
            "bout_r": bout[None, :].astype(bf),
            "iota": iota.astype(bf),
        })
    return in_maps, meta


def kernel(**inputs):
    nodes = np.asarray(inputs["nodes"])
    N = nodes.shape[0]
    LAYERS = np.asarray(inputs["Ws"]).shape[0]
    OUT = np.asarray(inputs["W_out"]).shape[1]
    assert N % NC == 0
    SH = N // NC
    SHP = (SH + P - 1) // P * P
    NB = SHP // P
    assert SHP <= 32767, "int16 gather index limit"
    assert NB % SB == 0

    in_maps, meta = _make_in_maps(inputs, N, SH, SHP, NB, LAYERS, OUT)

    import hashlib
    h = hashlib.sha1(meta["caps"].tobytes()).hexdigest()[:12]
    key = (NB, SHP, LAYERS, OUT, h)
    if key not in _CACHE:
        _CACHE[key] = _build_program(NB, SHP, LAYERS, OUT, meta)
    nc_prog = _CACHE[key]

    res, exec_info = _run(nc_prog, in_maps)
    global _LAST_EXEC
    _LAST_EXEC = exec_info
    out = np.concatenate([res[c]["out_own"][:SH] for c in range(NC)], axis=0)
    return out.astype(np.float32)



# revision 33
# speedup vs baseline: 1.1302x; 1.0389x over previous
"""GNN message-passing (GCN-mean) kernel for 8 Trainium2 NeuronCores. (V6)

V3 (diagonal slotting) plus:
- K0=2 with overflow edges pooled per (superblock, group) into statically
  ranged cross-block chunks (90% slot fill vs 76%), cutting gather
  descriptors ~15%.
- pad slots spread across all SHP-SH zero rows (same-row descriptors
  measured ~7x slower on HBM).
- overflow edges src-sorted within each pool for HBM locality.
- one-hot masks built in ONE batched DVE op per block (broadcast APs),
  emitted two blocks ahead so PE never stalls on mask builds.
- aggregation accumulated TRANSPOSED (paggT[feat, dst] via lhsT=x chunks,
  rhs=identity/one-hot), which removes the per-block PE transpose + ACT
  copy; bias applied as outer(deg+1, b) so the 1/(deg+1) scale folds into
  the ReLU's per-partition scale; nodes supplied pre-transposed by host.
- PSUM banks rebalanced: 4 bufs for the (long) aggregation accumulator,
  2 for the weight-matmul accumulator, 1 each for the final-layer tiles.
"""
import math
import os
import numpy as np

NC = 8
P = 128
EPS = 1e-5
SB = 7        # dst blocks per superblock
NCHUNK = 7
K0 = 2        # diagonal chunks per (block, group)

_CACHE = {}
_LAST_EXEC = None


# --------------------------------------------------------------------------
# device program
# --------------------------------------------------------------------------
def _build_program(NB, SHP, LAYERS, OUT, meta):
    import concourse.bacc as bacc
    import concourse.mybir as mybir
    import concourse.tile as tile
    from concourse.masks import make_identity

    NSUP = NB // SB
    SUPC = NSUP // NCHUNK
    BLKC = NB // NCHUNK
    XCOLS = meta["XCOLS"]
    DCOLS = meta["DCOLS"]
    IDXC = meta["IDXC"]
    OHMAX = meta["OHMAX"]
    per_sg = meta["per_sg"]          # [NSUP, 8] chunks per gather
    g_off = meta["g_off"]            # [NSUP, 8]
    reg_off = meta["reg_off"]        # [NSUP, SB, 8] diag col base
    pairs = meta["pairs"]            # [s][bl] -> [(g, xg col, dst col), ...]
    f32 = mybir.dt.float32
    bf16 = mybir.dt.bfloat16
    Alu = mybir.AluOpType
    Act = mybir.ActivationFunctionType

    nc = bacc.Bacc("TRN2", target_bir_lowering=False, num_devices=NC,
                   num_swdge_queues=4)

    nodes_bf = nc.declare_dram_parameter("nodes_bf", [P, SHP], bf16, isOutput=False)
    idxs = nc.declare_dram_parameter("idxs", [NSUP, P, IDXC], mybir.dt.int16, isOutput=False)
    dstrel = nc.declare_dram_parameter("dstrel", [NSUP, P, DCOLS], bf16, isOutput=False)
    invp = nc.declare_dram_parameter("invp", [P, NB], f32, isOutput=False)
    deg1p = nc.declare_dram_parameter("deg1p", [1, NB * P], bf16, isOutput=False)
    maskp = nc.declare_dram_parameter("maskp", [P, 1], f32, isOutput=False)
    W_in = nc.declare_dram_parameter("W_in", [P, P], bf16, isOutput=False)
    Wl = nc.declare_dram_parameter("Wl", [LAYERS, P, P], bf16, isOutput=False)
    Wout = nc.declare_dram_parameter("Wout", [P, OUT], bf16, isOutput=False)
    b_in_r = nc.declare_dram_parameter("b_in_r", [1, P], bf16, isOutput=False)
    bl_r = nc.declare_dram_parameter("bl_r", [LAYERS, 1, P], bf16, isOutput=False)
    bout_r = nc.declare_dram_parameter("bout_r", [1, OUT], bf16, isOutput=False)
    iota_in = nc.declare_dram_parameter("iota", [P, P], bf16, isOutput=False)
    out_own = nc.declare_dram_parameter("out_own", [SHP, OUT], f32, isOutput=True)

    rg = [list(range(NC))]

    with tile.TileContext(nc) as tc:
        with (
            tc.tile_pool(name="const", bufs=1) as cpool,
            tc.tile_pool(name="dram", bufs=1, space="DRAM") as dpool,
            tc.tile_pool(name="work", bufs=6) as wpool,
            tc.tile_pool(name="ohp", bufs=5) as ohpool,
            tc.tile_pool(name="idxp", bufs=3) as ipool,
            tc.tile_pool(name="xgp", bufs=2) as xgpool,
            tc.tile_pool(name="psum", bufs=4, space="PSUM") as ppool,
            tc.tile_pool(name="psumh", bufs=2, space="PSUM") as ppoolh,
            tc.tile_pool(name="psum1", bufs=1, space="PSUM") as ppool1,
        ):
            iota_t = cpool.tile([P, P], bf16)
            nc.sync.dma_start(out=iota_t[:], in_=iota_in[:])
            ident_bf = cpool.tile([P, P], bf16)
            make_identity(nc, ident_bf[:])
            eps_t = cpool.tile([P, 1], f32)
            nc.vector.memset(eps_t[:], EPS)
            W_in_t = cpool.tile([P, P], bf16)
            nc.sync.dma_start(out=W_in_t[:], in_=W_in[:])
            Wout_t = cpool.tile([P, OUT], bf16)
            nc.sync.dma_start(out=Wout_t[:], in_=Wout[:])
            ones1 = cpool.tile([1, P], bf16)
            nc.vector.memset(ones1[:], 1.0)
            bin_t = cpool.tile([1, P], bf16)
            nc.sync.dma_start(out=bin_t[:], in_=b_in_r[:])
            bl_t = []
            for l in range(LAYERS):
                t = cpool.tile([1, P], bf16, name=f"bl{l}")
                nc.sync.dma_start(out=t[:], in_=bl_r[l])
                bl_t.append(t)
            Wl_ts = []
            for l in range(LAYERS):
                t = cpool.tile([P, P], bf16, name=f"wl{l}")
                nc.sync.dma_start(out=t[:], in_=Wl[l])
                Wl_ts.append(t)
            bout_t = cpool.tile([1, OUT], bf16)
            nc.sync.dma_start(out=bout_t[:], in_=bout_r[:])
            inv_t = cpool.tile([P, NB], f32)
            nc.sync.dma_start(out=inv_t[:], in_=invp[:])
            mask_t = cpool.tile([P, 1], f32)
            nc.sync.dma_start(out=mask_t[:], in_=maskp[:])

            ag_in = [dpool.tile([SHP, P], bf16, name=f"ag_in{l}") for l in range(LAYERS)]
            x_full = [
                dpool.tile([NC, SHP, P], bf16, addr_space="Shared", name=f"x_full{l}")
                for l in range(LAYERS)
            ]

            def emit_ag(l_t, k):
                nc.gpsimd.collective_compute(
                    "AllGather", Alu.bypass, replica_groups=rg,
                    ins=[ag_in[l_t][:].opt()],
                    outs=[x_full[l_t][:].opt()],
                )

            # ---------------- Phase A: x0 = nodes @ W_in + b_in ----------
            for b in range(NB):
                nbT = wpool.tile([P, P], bf16, tag="nb")
                nc.sync.dma_start(out=nbT[:], in_=nodes_bf[:, b * P:(b + 1) * P])
                ph = ppoolh.tile([P, P], f32, tag="h", space="PSUM")
                nc.tensor.matmul(ph[:], lhsT=nbT[:], rhs=W_in_t[:], start=True, stop=False)
                nc.tensor.matmul(ph[:], lhsT=ones1[:], rhs=bin_t[:], start=False, stop=True)
                x0b = wpool.tile([P, P], bf16, tag="xnext")
                if b == NB - 1:
                    # zero the pad node rows so the gather zero-row stays 0
                    nc.scalar.activation(x0b[:], ph[:], Act.Copy, scale=mask_t[:, 0:1])
                else:
                    nc.scalar.copy(out=x0b[:], in_=ph[:])
                nc.sync.dma_start(out=ag_in[0][b * P:(b + 1) * P, :], in_=x0b[:])
            emit_ag(0, 0)

            # ---------------- Layers ------------------------------------
            for l in range(LAYERS):
                xf = x_full[l]
                last = l == LAYERS - 1
                for s in range(NSUP):
                    idx_t = ipool.tile([P, IDXC], mybir.dt.int16, tag="idx")
                    nc.sync.dma_start(out=idx_t[:], in_=idxs[s])
                    dst_t = ipool.tile([P, DCOLS], bf16, tag="dst")
                    nc.sync.dma_start(out=dst_t[:], in_=dstrel[s])
                    deg1_t = ipool.tile([1, SB * P], bf16, tag="deg1")
                    nc.sync.dma_start(out=deg1_t[:],
                                      in_=deg1p[0:1, s * SB * P:(s + 1) * SB * P])
                    xg = xgpool.tile([P, XCOLS, P], bf16, tag="xg")
                    for g in range(8):
                        cols = int(per_sg[s, g])
                        co = int(g_off[s, g])
                        # split into two half-calls on different queues to
                        # keep all 4 SWDGE queues busy within a superblock
                        h1 = cols // 2
                        for hi, (c0, cn) in enumerate(((co, h1), (co + h1, cols - h1))):
                            nci = cn * P
                            nc.gpsimd.dma_gather(
                                out_ap=xg[:, c0:c0 + cn, :],
                                in_ap=xf[g],
                                idxs_ap=idx_t[:, c0 * 8:c0 * 8 + nci // 16],
                                num_idxs=nci,
                                num_idxs_reg=nci,
                                elem_size=P,
                                single_packet=False,
                                queue_num=(2 * g + hi) % 4,
                            )
                    def build_oh(bl):
                        # one batched DVE op builds all of a block's one-hots
                        prs_b = pairs[s][bl]
                        t = ohpool.tile([P, OHMAX, P], bf16, tag="oh")
                        if prs_b:
                            noh_b = len(prs_b)
                            dc0 = prs_b[0][2]
                            nc.vector.tensor_tensor(
                                out=t[:, 0:noh_b, :],
                                in0=iota_t[:].unsqueeze(1).to_broadcast([P, noh_b, P]),
                                in1=dst_t[:, dc0:dc0 + noh_b].unsqueeze(2)
                                    .to_broadcast([P, noh_b, P]),
                                op=Alu.is_equal,
                            )
                        return t

                    oh_q = [build_oh(0), build_oh(1)]
                    for bl_i in range(SB):
                        b = s * SB + bl_i
                        prs = pairs[s][bl_i]   # [(g, xg col, dst col), ...]
                        oh = oh_q.pop(0)
                        if bl_i + 2 < SB:
                            oh_q.append(build_oh(bl_i + 2))
                        xs_bf = wpool.tile([P, P], bf16, tag="xs")
                        nc.sync.dma_start(out=xs_bf[:], in_=ag_in[l][b * P:(b + 1) * P, :])
                        noh = len(prs)
                        # aggregate TRANSPOSED: paggT[feat, dst] avoids the
                        # per-block PE transpose + ACT copy before the W matmul
                        paggT = ppool.tile([P, P], f32, tag="agg", space="PSUM")
                        nmm = 1 + 8 * K0 + noh
                        nc.tensor.matmul(paggT[:], lhsT=xs_bf[:], rhs=ident_bf[:],
                                         start=True, stop=(nmm == 1))
                        mm = 1
                        for g in range(8):
                            ro = int(reg_off[s, bl_i, g])
                            for k in range(K0):
                                mm += 1
                                nc.tensor.matmul(
                                    paggT[:], lhsT=xg[:, ro + k, :], rhs=ident_bf[:],
                                    start=False, stop=(mm == nmm),
                                )
                        for j, (g_, col_, dc_) in enumerate(prs):
                            mm += 1
                            nc.tensor.matmul(
                                paggT[:], lhsT=xg[:, col_, :], rhs=oh[:, j, :],
                                start=False, stop=(mm == nmm),
                            )
                        # ---- fused epilogue ----
                        m1T = wpool.tile([P, P], bf16, tag="m1")
                        nc.scalar.copy(out=m1T[:], in_=paggT[:])
                        ph = ppoolh.tile([P, P], f32, tag="h", space="PSUM")
                        nc.tensor.matmul(ph[:], lhsT=m1T[:], rhs=Wl_ts[l][:],
                                         start=True, stop=False)
                        # bias as outer((deg+1), b): inv scale below folds it to +b
                        nc.tensor.matmul(ph[:], lhsT=deg1_t[0:1, bl_i * P:(bl_i + 1) * P],
                                         rhs=bl_t[l][:], start=False, stop=True)
                        hr = wpool.tile([P, P], f32, tag="hr")
                        mu_s = wpool.tile([P, 1], f32, tag="mus")
                        nc.scalar.activation(hr[:], ph[:], Act.Relu,
                                             scale=inv_t[:, b:b + 1], accum_out=mu_s[:])
                        h2 = wpool.tile([P, P], f32, tag="h2")
                        s2 = wpool.tile([P, 1], f32, tag="s2")
                        nc.scalar.activation(h2[:], hr[:], Act.Square, accum_out=s2[:])
                        musq = wpool.tile([P, 1], f32, tag="musq")
                        nc.scalar.activation(musq[:], mu_s[:], Act.Square,
                                             scale=1.0 / P)
                        var2 = wpool.tile([P, 1], f32, tag="var2")
                        nc.vector.scalar_tensor_tensor(
                            out=var2[:], in0=s2[:], scalar=1.0 / P, in1=musq[:],
                            op0=Alu.mult, op1=Alu.subtract,
                        )
                        std_t = wpool.tile([P, 1], f32, tag="std")
                        nc.scalar.activation(std_t[:], var2[:], Act.Sqrt, bias=eps_t[:, 0:1])
                        rstd = wpool.tile([P, 1], f32, tag="rstd")
                        nc.vector.reciprocal_approx_fast(out=rstd[:], in_=std_t[:])
                        mu_t = wpool.tile([P, 1], f32, tag="mu")
                        nc.scalar.activation(mu_t[:], mu_s[:], Act.Copy,
                                             scale=1.0 / P)
                        y_bf = wpool.tile([P, P], bf16, tag="xnext" if not last else "yf")
                        nc.vector.tensor_scalar(
                            out=y_bf[:], in0=hr[:], scalar1=mu_t[:, 0:1],
                            scalar2=rstd[:, 0:1], op0=Alu.subtract, op1=Alu.mult,
                        )
                        if not last:
                            if b == NB - 1:
                                ym = wpool.tile([P, P], bf16, tag="ym")
                                nc.vector.tensor_scalar(
                                    out=ym[:], in0=y_bf[:], scalar1=mask_t[:, 0:1],
                                    scalar2=None, op0=Alu.mult,
                                )
                                nc.sync.dma_start(out=ag_in[l + 1][b * P:(b + 1) * P, :], in_=ym[:])
                            else:
                                nc.sync.dma_start(out=ag_in[l + 1][b * P:(b + 1) * P, :], in_=y_bf[:])
                        else:
                            pyT = ppool1.tile([P, P], bf16, tag="mT", space="PSUM")
                            nc.tensor.transpose(pyT[:], y_bf[:], ident_bf[:])
                            yT = wpool.tile([P, P], bf16, tag="mTs")
                            nc.scalar.copy(out=yT[:], in_=pyT[:])
                            po = ppool1.tile([P, OUT], f32, tag="po", space="PSUM")
                            nc.tensor.matmul(po[:], lhsT=yT[:], rhs=Wout_t[:], start=True, stop=False)
                            nc.tensor.matmul(po[:], lhsT=ones1[:], rhs=bout_t[:], start=False, stop=True)
                            ob = wpool.tile([P, OUT], f32, tag="ob")
                            nc.scalar.copy(out=ob[:], in_=po[:])
                            nc.sync.dma_start(out=out_own[b * P:(b + 1) * P, :], in_=ob[:])
                if not last:
                    emit_ag(l + 1, 0)

    nc.compile()
    return nc


# --------------------------------------------------------------------------
# host-side sharding prep (diagonal slotting)
# --------------------------------------------------------------------------
def _prep_edges(src, dst, N, SH, SHP, NB):
    E = src.shape[0]
    src = src.astype(np.int64)
    dst = dst.astype(np.int64)
    NSUP = NB // SB
    ZR = SHP - 1  # zero row within each group table

    core = dst // SH
    dst_loc = dst - core * SH
    blk = dst_loc >> 7
    dst_rel = dst_loc & 127
    grp = src // SH
    src_loc = src - grp * SH

    key = ((core * NB + blk) * 8 + grp) * P + dst_rel
    order = np.lexsort((src_loc, key))
    ks = key[order]
    sl = src_loc[order]
    dr = dst_rel[order]

    cell_counts = np.bincount(ks, minlength=NC * NB * 8 * P)
    cell_starts = np.zeros_like(cell_counts)
    np.cumsum(cell_counts[:-1], out=cell_starts[1:])
    rank = np.arange(E, dtype=np.int64) - cell_starts[ks]

    cbg = ks // P
    d = ks % P

    is_diag = rank < K0
    ovf_mask = ~is_diag

    # overflow edges pooled per (superblock, group) with STATIC per-block
    # ranges (max over cores) so chunk->block structure is core-independent
    ovf_counts = np.bincount(cbg[ovf_mask], minlength=NC * NB * 8).reshape(NC, NB, 8)
    R_sb = ovf_counts.max(axis=0).reshape(NSUP, SB, 8)   # static range [s, bl, g]
    start_off = np.zeros((NSUP, SB, 8), np.int64)
    start_off[:, 1:, :] = np.cumsum(R_sb, axis=1)[:, :-1, :]
    pool = R_sb.sum(axis=1)                              # [s, g]
    novf_chunks = -(-pool // P)                          # [s, g]

    per_sg = SB * K0 + novf_chunks                       # [s, g] cols per gather
    g_off = np.zeros((NSUP, 8), np.int64)
    g_off[:, 1:] = np.cumsum(per_sg[:, :-1], axis=1)
    reg_off = (g_off[:, None, :] +
               (np.arange(SB) * K0)[None, :, None])      # diag col base [s, bl, g]
    ovf_base = g_off + SB * K0                           # first ovf col [s, g]
    XCOLS = int(per_sg.sum(axis=1).max())

    # (chunk, block) pair lists per (s, bl): static from ranges
    pairs = [[[] for _ in range(SB)] for _ in range(NSUP)]
    maxci = int(novf_chunks.max())
    dcol_lut = np.full((NSUP, 8, maxci, SB), -1, np.int64)
    DCOLS = 0
    for s in range(NSUP):
        dcol = 0
        for bl in range(SB):
            for gg in range(8):
                rn = int(R_sb[s, bl, gg])
                if rn == 0:
                    continue
                st = int(start_off[s, bl, gg])
                for ci in range(st >> 7, ((st + rn - 1) >> 7) + 1):
                    pairs[s][bl].append((gg, int(ovf_base[s, gg] + ci), dcol))
                    dcol_lut[s, gg, ci, bl] = dcol
                    dcol += 1
        DCOLS = max(DCOLS, dcol)
    OHMAX = max(len(pairs[s][bl]) for s in range(NSUP) for bl in range(SB))

    sup_e = (cbg // 8) % NB // SB
    b_local = ((cbg // 8) % NB) % SB
    g = cbg % 8
    c_ = cbg // (NB * 8)

    ovf_rank = np.zeros(E, np.int64)
    oc = np.bincount(cbg[ovf_mask], minlength=NC * NB * 8)
    os_ = np.zeros_like(oc)
    np.cumsum(oc[:-1], out=os_[1:])
    # rank overflow edges within their (core, block, group) pool in
    # src-ascending order for HBM gather locality
    ovf_idx = np.nonzero(ovf_mask)[0]
    o2 = np.lexsort((sl[ovf_idx], cbg[ovf_idx]))
    sorted_pos = ovf_idx[o2]
    ovf_rank[sorted_pos] = np.arange(len(ovf_idx)) - os_[cbg[sorted_pos]]

    pool_pos = start_off[sup_e, b_local, g] + ovf_rank
    chunk = np.where(
        is_diag,
        reg_off[sup_e, b_local, g] + rank,
        ovf_base[sup_e, g] + (pool_pos >> 7),
    )
    part = np.where(is_diag, d, pool_pos & 127)

    IDX_TOT = XCOLS * P
    # pad slots gather zero rows; spread across all SHP-SH zero rows to avoid
    # HBM same-address hotspotting (same-row descriptors measured ~7x slower)
    NZ = SHP - SH
    padpat = (SH + (np.arange(IDX_TOT) % NZ)).astype(np.int16)
    idx16 = np.broadcast_to(padpat, (NC, NSUP, IDX_TOT)).copy()
    slot = chunk * P + part
    idx16[c_, sup_e, slot] = sl.astype(np.int16)

    dstv = np.full((NC, NSUP, DCOLS, P), -1.0, np.float32)
    m = ovf_mask
    ocol = dcol_lut[sup_e[m], g[m], pool_pos[m] >> 7, b_local[m]]
    assert (ocol >= 0).all()
    dstv[c_[m], sup_e[m], ocol, part[m]] = dr[m].astype(np.float32)
    import ml_dtypes
    dst_dev = np.ascontiguousarray(dstv.transpose(0, 1, 3, 2)).astype(ml_dtypes.bfloat16)

    IDXC = IDX_TOT // 16
    A = idx16.reshape(NC, NSUP, IDXC, 16)
    Bm = A.transpose(0, 1, 3, 2)
    idx_dev = np.ascontiguousarray(
        np.broadcast_to(Bm[:, :, None, :, :], (NC, NSUP, 8, 16, IDXC))
        .reshape(NC, NSUP, P, IDXC)
    )
    meta = dict(caps=R_sb, per_sg=per_sg, g_off=g_off, reg_off=reg_off,
                pairs=pairs, XCOLS=XCOLS,
                DCOLS=DCOLS, IDXC=IDXC, OHMAX=OHMAX)
    return idx_dev, dst_dev, meta


def _run(nc_prog, in_maps):
    import jax
    import numpy as np
    from jax.sharding import Mesh, PartitionSpec, NamedSharding
    from jax.experimental.shard_map import shard_map
    import concourse.mybir as mybir
    from concourse.bass2jax import _bass_exec_p, install_neuronx_cc_hook, partition_id_tensor

    install_neuronx_cc_hook()
    nc = nc_prog
    partition_name = nc.partition_id_tensor.name if nc.partition_id_tensor else None
    in_names, out_names, out_avals, zero_outs = [], [], [], []
    for alloc in nc.m.functions[0].allocations:
        if not isinstance(alloc, mybir.MemoryLocationSet):
            continue
        name = alloc.memorylocations[0].name
        if alloc.kind == "ExternalInput":
            if name != partition_name:
                in_names.append(name)
        elif alloc.kind == "ExternalOutput":
            out_names.append(name)
            shape = tuple(alloc.tensor_shape)
            dtype = mybir.dt.np(alloc.dtype)
            out_avals.append(jax.core.ShapedArray(shape, dtype))
            zero_outs.append(np.zeros(shape, dtype))
    n_params = len(in_names)
    all_in = list(in_names) + list(out_names)
    if partition_name is not None:
        all_in.append(partition_name)

    def _body(*args):
        operands = list(args)
        if partition_name is not None:
            operands.append(partition_id_tensor())
        outs = _bass_exec_p.bind(
            *operands,
            out_avals=tuple(out_avals),
            in_names=tuple(all_in),
            out_names=tuple(out_names),
            lowering_input_output_aliases=(),
            sim_require_finite=False,
            sim_require_nnan=False,
            nc=nc,
        )
        return tuple(outs)

    devices = jax.devices()[:NC]
    mesh = Mesh(np.asarray(devices), ("core",))
    in_specs = (PartitionSpec("core"),) * (n_params + len(out_names))
    out_specs = (PartitionSpec("core"),) * len(out_names)
    fn = jax.jit(
        shard_map(_body, mesh=mesh, in_specs=in_specs, out_specs=out_specs,
                  check_rep=False),
        keep_unused=True,
    )
    concat_in = [
        np.concatenate([np.asarray(in_maps[c][k]) for c in range(NC)], axis=0)
        for k in in_names
    ]
    concat_zero = [np.zeros((NC * z.shape[0], *z.shape[1:]), z.dtype) for z in zero_outs]
    sharding = NamedSharding(mesh, PartitionSpec("core"))
    dev_in = [jax.device_put(a, sharding) for a in concat_in + concat_zero]
    outs = fn(*dev_in)
    jax.block_until_ready(outs)
    res = [
        {name: np.asarray(outs[i]).reshape(NC, *out_avals[i].shape)[c]
         for i, name in enumerate(out_names)}
        for c in range(NC)
    ]
    return res, (fn, dev_in, out_names, out_avals)


def _make_in_maps(inputs, N, SH, SHP, NB, LAYERS, OUT):
    import ml_dtypes
    bf = ml_dtypes.bfloat16
    nodes = np.asarray(inputs["nodes"], np.float32)
    src = np.asarray(inputs["src"])
    dst = np.asarray(inputs["dst"])
    W_in = np.asarray(inputs["W_in"], np.float32)
    b_in = np.asarray(inputs["b_in"], np.float32)
    Ws = np.asarray(inputs["Ws"], np.float32)
    bs = np.asarray(inputs["bs"], np.float32)
    gammas = np.asarray(inputs["gammas"], np.float32)
    betas = np.asarray(inputs["betas"], np.float32)
    W_out = np.asarray(inputs["W_out"], np.float32)
    b_out = np.asarray(inputs["b_out"], np.float32)

    idx_dev, dst_dev, meta = _prep_edges(src, dst, N, SH, SHP, NB)

    deg = np.bincount(dst, minlength=N).astype(np.float32)
    inv = 1.0 / (deg + 1.0)
    invp = np.ones((NC, SHP), np.float32)
    invp.reshape(NC, SHP)[:, :SH] = inv.reshape(NC, SH)
    deg1 = np.ones((NC, SHP), np.float32)
    deg1.reshape(NC, SHP)[:, :SH] = (deg + 1.0).reshape(NC, SH)
    assert deg1.max() <= 256, "deg+1 must be bf16-exact"
    invp = np.ascontiguousarray(invp.reshape(NC, NB, P).transpose(0, 2, 1))

    nvalid = SH - (NB - 1) * P
    maskp = (np.arange(P) < nvalid).astype(np.float32)[:, None]

    Wl = np.zeros((LAYERS, P, P), np.float32)
    bl = np.zeros((LAYERS, P), np.float32)
    Wl[0] = Ws[0]
    bl[0] = bs[0]
    for l in range(1, LAYERS):
        Wl[l] = gammas[l - 1][:, None] * Ws[l]
        bl[l] = betas[l - 1] @ Ws[l] + bs[l]
    Wout = gammas[LAYERS - 1][:, None] * W_out
    bout = betas[LAYERS - 1] @ W_out + b_out

    iota = np.tile(np.arange(P, dtype=np.float32), (P, 1))

    in_maps = []
    for c in range(NC):
        nsh = np.zeros((SHP, P), bf)
        nsh[:SH] = nodes[c * SH:(c + 1) * SH].astype(bf)
        in_maps.append({
            "nodes_bf": np.ascontiguousarray(nsh.T),
            "idxs": idx_dev[c],
            "dstrel": dst_dev[c],
            "invp": invp[c],
            "deg1p": deg1[c][None, :].astype(bf),
            "maskp": maskp,
            "W_in": W_in.astype(bf),
            "Wl": Wl.astype(bf),
            "Wout": Wout.astype(bf),
            "b_in_r": b_in[None, :].astype(bf),
            "bl_r": bl[:, None, :].astype(bf),
            "bout_r": bout[None, :].astype(bf),
            "iota": iota.astype(bf),
        })
    return in_maps, meta


def kernel(**inputs):
    nodes = np.asarray(inputs["nodes"])
    N = nodes.shape[0]
    LAYERS = np.asarray(inputs["Ws"]).shape[0]
    OUT = np.asarray(inputs["W_out"]).shape[1]
    assert N % NC == 0
    SH = N // NC
    SHP = (SH + P - 1) // P * P
    NB = SHP // P
    assert SHP <= 32767, "int16 gather index limit"
    assert NB % SB == 0

    in_maps, meta = _make_in_maps(inputs, N, SH, SHP, NB, LAYERS, OUT)

    import hashlib
    h = hashlib.sha1(meta["caps"].tobytes()).hexdigest()[:12]
    key = (NB, SHP, LAYERS, OUT, h)
    if key not in _CACHE:
        _CACHE[key] = _build_program(NB, SHP, LAYERS, OUT, meta)
    nc_prog = _CACHE[key]

    res, exec_info = _run(nc_prog, in_maps)
    global _LAST_EXEC
    _LAST_EXEC = exec_info
    out = np.concatenate([res[c]["out_own"][:SH] for c in range(NC)], axis=0)
    return out.astype(np.float32)

